# revision 8
# baseline (speedup 1.0000x reference)
"""Bass/Tile TRN2 kernel for nn_LzScaleDotAttention (B=8, L=2048, D=512).

Math per batch b (see module docstring of the nn problem):
    S[q,k]   = sum_d Q[q,d] K[k,d]
    E        = exp(S)                       # inputs are pre-scaled small, no max-sub needed
    num[k,d] = sum_q E[q,k] V[q,d]          # = E^T @ V
    den[k]   = sum_q E[q,k]
    mask[k]  = 1.0 if any(V[k,:] != 0) else 0.0
    out[k,d] = num[k,d] * mask[k]*c / (den[k]*mask[k]*c + EPS),  c = 1/sqrt(D)

The renormalisation over the query axis commutes with the E^T@V contraction
(the divisor depends only on k), so we never materialise the normalised
attention matrix: one flash-style pass over q tiles accumulates num (PSUM)
and den (SBUF f32 accumulator + one tiny cross-partition matmul with ones).

Sharding: batch dim (8) across the 8 NeuronCores, one batch per core (SPMD,
no collectives). Matmuls run in float32r (full fp32 storage, fast PE mode).
"""

import math
import os
import sys

import numpy as np

for _p in ("/opt/trn_rl_repo", "/root/.axon_site/_ro/trn_rl_repo"):
    if os.path.isdir(_p) and _p not in sys.path:
        sys.path.append(_p)

import concourse.bacc as bacc
import concourse.mybir as mybir
import concourse.tile as tile
from concourse.bass import ds, ts
from concourse.bass_utils import run_bass_kernel_spmd
from concourse.masks import make_identity

B, L, D = 8, 2048, 512
P = 128
EPS = 1e-7
N_CORES = 8

f32 = mybir.dt.float32
f32r = mybir.dt.float32r
AF = mybir.ActivationFunctionType
ALU = mybir.AluOpType


def _r(ap):
    """fp32r view: same bits as fp32, runs the PE at full rate for N>=256."""
    return ap.bitcast(f32r)


def build_program(Lb=L, Db=D, n_cores=N_CORES):
    NT = Lb // P          # 128-row tiles along q / k timesteps
    DC = Db // P          # 128-wide chunks of the feature dim
    KBW = 512             # k-block width (one PSUM bank of fp32)
    KB = Lb // KBW        # k blocks
    KT = KBW // P         # 128-wide k tiles per block
    C = 1.0 / math.sqrt(Db)

    nc = bacc.Bacc(
        "TRN2", target_bir_lowering=False, debug=False, num_devices=n_cores
    )
    q = nc.dram_tensor("q", [Lb, Db], f32, kind="ExternalInput").ap()
    k = nc.dram_tensor("k", [Lb, Db], f32, kind="ExternalInput").ap()
    v = nc.dram_tensor("v", [Lb, Db], f32, kind="ExternalInput").ap()
    out = nc.dram_tensor("out", [Lb, Db], f32, kind="ExternalOutput").ap()

    with tile.TileContext(nc) as tc:
        with (
            tc.tile_pool(name="const", bufs=1) as cpool,
            tc.tile_pool(name="qTp", bufs=NT) as qT_pool,
            tc.tile_pool(name="kTp", bufs=KB) as kT_pool,
            tc.tile_pool(name="vSp", bufs=NT) as vS_pool,
            tc.tile_pool(name="stage", bufs=6) as stage_pool,
            tc.tile_pool(name="warm", bufs=1) as warm_pool,
            tc.tile_pool(name="ep", bufs=3) as e_pool,
            tc.tile_pool(name="accp", bufs=2) as acc_pool,
            tc.tile_pool(name="outp", bufs=3) as out_pool,
            tc.tile_pool(name="scp", bufs=4) as sc_pool,
            tc.tile_pool(name="ps_s", bufs=2, space="PSUM") as ps_s,
            tc.tile_pool(name="ps_num", bufs=1, space="PSUM") as ps_num,
            tc.tile_pool(name="ps_tp", bufs=2, space="PSUM") as ps_tp,
        ):
            ident = cpool.tile([P, P], f32, name="ident")
            make_identity(nc, ident)
            ones = cpool.tile([P, 1], f32, name="ones")
            nc.vector.memset(ones, 1.0)
            vmask = cpool.tile([P, NT], f32, name="vmask")

            # PE warm-up: ~3.4us of sustained dummy matmul activity flips the
            # HAM clock gate to full rate before the real work arrives
            # (plain fp32: 4 cycles/row, few instructions, no f32r rounding
            # rule on the zero inputs)
            zf = warm_pool.tile([P, KBW], f32, name="zf")
            nc.vector.memset(zf, 0.0)
            for w in range(6):
                wps = ps_tp.tile([P, KBW], f32, tag="tp", name=f"wps{w}")
                nc.tensor.matmul(
                    wps, zf[:, :P], zf, start=True, stop=True
                )

            # Persistent SBUF residents: Q^T, K^T (feature-major) and V.
            qT_t = [
                qT_pool.tile([P, DC, P], f32r, tag="qT", name=f"qT{t}")
                for t in range(NT)
            ]
            kT_b = [
                kT_pool.tile([P, DC, KBW], f32r, tag="kT", name=f"kTb{b}")
                for b in range(KB)
            ]
            vS_t = [
                vS_pool.tile([P, Db], f32r, tag="vS", name=f"vS{t}")
                for t in range(NT)
            ]

            # ---- Phase 0: load V; load + PE-transpose Q and K ----
            for t in range(NT):
                # gpsimd DMA may cast (relabel) f32 -> f32r; the PE rounds on
                # ingest anyway, so no separate rounding pass is needed
                nc.gpsimd.dma_start(vS_t[t], v[ts(t, P), :])
                nc.vector.tensor_reduce(
                    vmask[:, t : t + 1],
                    vS_t[t],
                    axis=mybir.AxisListType.X,
                    op=ALU.max,
                    apply_absolute_value=True,
                )
                kf = stage_pool.tile([P, Db], f32, tag="kf", name=f"kf{t}")
                nc.sync.dma_start(kf, k[ts(t, P), :])
                qf = stage_pool.tile([P, Db], f32, tag="qf", name=f"qf{t}")
                nc.sync.dma_start(qf, q[ts(t, P), :])
                for dc in range(DC):
                    tpk = ps_tp.tile([P, P], f32, tag="tp", name=f"tpk{t}_{dc}")
                    nc.tensor.transpose(tpk, kf[:, ts(dc, P)], ident)
                    nc.scalar.copy(kT_b[t // KT][:, dc, ts(t % KT, P)], tpk)
                    tpq = ps_tp.tile([P, P], f32, tag="tp", name=f"tpq{t}_{dc}")
                    nc.tensor.transpose(tpq, qf[:, ts(dc, P)], ident)
                    nc.vector.tensor_copy(qT_t[t][:, dc, :], tpq)
            # mask[k] = (max_d |v[k,d]|) > 0  ->  {0.0, 1.0}; pm = mask * c
            nc.vector.tensor_scalar(vmask, vmask, 0.0, None, op0=ALU.is_gt)
            pm = cpool.tile([P, NT], f32, name="pm")
            nc.vector.tensor_scalar_mul(pm, vmask, C)

            # ---- Main flash loop over k blocks ----
            for kb in range(KB):
                nums = [
                    ps_num.tile([P, Db], f32, tag=f"num{kt}", name=f"num{kb}_{kt}")
                    for kt in range(KT)
                ]
                acc = acc_pool.tile([P, KBW], f32, tag="acc", name=f"acc{kb}")
                e_tiles = {}
                # software pipeline: stage-1 (scores+exp) runs one q-tile
                # ahead of stage-2 (E^T @ V) so the PE never waits on ACT
                for qt in range(NT + 1):
                    if qt < NT:
                        s_ps = ps_s.tile([P, KBW], f32, tag="s", name=f"s{kb}_{qt}")
                        for dc in range(DC):
                            nc.tensor.matmul(
                                s_ps,
                                qT_t[qt][:, dc, :],
                                kT_b[kb][:, dc, :],
                                start=(dc == 0),
                                stop=(dc == DC - 1),
                            )
                        e = e_pool.tile([P, KBW], f32r, tag="e", name=f"e{kb}_{qt}")
                        nc.scalar.activation(e, s_ps, AF.Exp)
                        if qt == 0:
                            nc.vector.tensor_copy(acc, e)
                        else:
                            nc.vector.tensor_add(acc, acc, e)
                        e_tiles[qt] = e
                    if qt >= 1:
                        ep = e_tiles.pop(qt - 1)
                        for kt in range(KT):
                            nc.tensor.matmul(
                                nums[kt],
                                ep[:, ts(kt, P)],
                                vS_t[qt - 1],
                                start=(qt - 1 == 0),
                                stop=(qt - 1 == NT - 1),
                            )
                # close the k block: den, scale, writeback
                for kt in range(KT):
                    j = kb * KT + kt
                    dps = ps_tp.tile([P, 1], f32, tag="tp", name=f"dps{j}")
                    nc.tensor.matmul(
                        dps, acc[:, ts(kt, P)], ones, start=True, stop=True
                    )
                    # scale = pm / (den * pm + EPS), pm = mask/sqrt(D)
                    scl = sc_pool.tile([P, 1], f32, tag="scl", name=f"scl{j}")
                    nc.vector.tensor_scalar(
                        scl, dps, pm[:, j : j + 1], EPS, op0=ALU.mult, op1=ALU.add
                    )
                    rcp = sc_pool.tile([P, 1], f32, tag="rcp", name=f"rcp{j}")
                    nc.vector.reciprocal(rcp, scl)
                    nc.vector.tensor_mul(rcp, rcp, pm[:, j : j + 1])
                    o = out_pool.tile([P, Db], f32, tag="o", name=f"o{j}")
                    nc.scalar.mul(o, nums[kt], rcp)
                    nc.sync.dma_start(out[ts(j, P), :], o)

    return nc


_cache = {}


def _get_compiled(Lb=L, Db=D):
    key = (Lb, Db)
    if key not in _cache:
        nc = build_program(Lb, Db)
        nc.compile()
        _cache[key] = nc
    return _cache[key]


def run(q, k, v, trace=False):
    nc = _get_compiled()
    q = np.ascontiguousarray(q, dtype=np.float32)
    k = np.ascontiguousarray(k, dtype=np.float32)
    v = np.ascontiguousarray(v, dtype=np.float32)
    in_maps = [
        {"q": q[i], "k": k[i], "v": v[i]} for i in range(N_CORES)
    ]
    res = run_bass_kernel_spmd(nc, in_maps, list(range(N_CORES)), trace=trace)
    out = np.stack([res.results[i]["out"] for i in range(N_CORES)], axis=0)
    return out.astype(np.float32, copy=False), res


def kernel(q, k, v):
    out, _ = run(q, k, v, trace=False)
    return out


# revision 9
# speedup vs baseline: 1.0496x; 1.0496x over previous
"""Bass/Tile TRN2 kernel for nn_LzScaleDotAttention (B=8, L=2048, D=512).

Math per batch b (see module docstring of the nn problem):
    S[q,k]   = sum_d Q[q,d] K[k,d]
    E        = exp(S)                       # inputs are pre-scaled small, no max-sub needed
    num[k,d] = sum_q E[q,k] V[q,d]          # = E^T @ V
    den[k]   = sum_q E[q,k]
    mask[k]  = 1.0 if any(V[k,:] != 0) else 0.0
    out[k,d] = num[k,d] * mask[k]*c / (den[k]*mask[k]*c + EPS),  c = 1/sqrt(D)

The renormalisation over the query axis commutes with the E^T@V contraction
(the divisor depends only on k), so we never materialise the normalised
attention matrix: one flash-style pass over q tiles accumulates num (PSUM)
and den (SBUF f32 accumulator + one tiny cross-partition matmul with ones).

Sharding: batch dim (8) across the 8 NeuronCores, one batch per core (SPMD,
no collectives). Matmuls run in float32r (full fp32 storage, fast PE mode).
"""

import math
import os
import sys

import numpy as np

for _p in ("/opt/trn_rl_repo", "/root/.axon_site/_ro/trn_rl_repo"):
    if os.path.isdir(_p) and _p not in sys.path:
        sys.path.append(_p)

import concourse.bacc as bacc
import concourse.mybir as mybir
import concourse.tile as tile
from concourse.bass import ds, ts
from concourse.bass_utils import run_bass_kernel_spmd
from concourse.masks import make_identity

B, L, D = 8, 2048, 512
P = 128
EPS = 1e-7
N_CORES = 8

f32 = mybir.dt.float32
f32r = mybir.dt.float32r
AF = mybir.ActivationFunctionType
ALU = mybir.AluOpType


def _r(ap):
    """fp32r view: same bits as fp32, runs the PE at full rate for N>=256."""
    return ap.bitcast(f32r)


def build_program(Lb=L, Db=D, n_cores=N_CORES):
    NT = Lb // P          # 128-row tiles along q / k timesteps
    DC = Db // P          # 128-wide chunks of the feature dim
    KBW = 512             # k-block width (one PSUM bank of fp32)
    KB = Lb // KBW        # k blocks
    KT = KBW // P         # 128-wide k tiles per block
    C = 1.0 / math.sqrt(Db)

    nc = bacc.Bacc(
        "TRN2", target_bir_lowering=False, debug=False, num_devices=n_cores
    )
    q = nc.dram_tensor("q", [Lb, Db], f32, kind="ExternalInput").ap()
    k = nc.dram_tensor("k", [Lb, Db], f32, kind="ExternalInput").ap()
    v = nc.dram_tensor("v", [Lb, Db], f32, kind="ExternalInput").ap()
    out = nc.dram_tensor("out", [Lb, Db], f32, kind="ExternalOutput").ap()

    with tile.TileContext(nc) as tc:
        with (
            tc.tile_pool(name="const", bufs=1) as cpool,
            tc.tile_pool(name="qTp", bufs=NT) as qT_pool,
            tc.tile_pool(name="kTp", bufs=KB) as kT_pool,
            tc.tile_pool(name="vSp", bufs=NT) as vS_pool,
            tc.tile_pool(name="stage", bufs=6) as stage_pool,
            tc.tile_pool(name="warm", bufs=1) as warm_pool,
            tc.tile_pool(name="ep", bufs=3) as e_pool,
            tc.tile_pool(name="accp", bufs=2) as acc_pool,
            tc.tile_pool(name="outp", bufs=3) as out_pool,
            tc.tile_pool(name="scp", bufs=4) as sc_pool,
            tc.tile_pool(name="ps_s", bufs=2, space="PSUM") as ps_s,
            tc.tile_pool(name="ps_num", bufs=1, space="PSUM") as ps_num,
            tc.tile_pool(name="ps_tp", bufs=2, space="PSUM") as ps_tp,
        ):
            ident = cpool.tile([P, P], f32, name="ident")
            make_identity(nc, ident)
            ones = cpool.tile([P, 1], f32, name="ones")
            nc.vector.memset(ones, 1.0)
            vmask = cpool.tile([P, NT], f32, name="vmask")

            # PE warm-up: ~3.4us of sustained dummy matmul activity flips the
            # HAM clock gate to full rate before the real work arrives
            # (plain fp32: 4 cycles/row, few instructions, no f32r rounding
            # rule on the zero inputs)
            zf = warm_pool.tile([P, KBW], f32, name="zf")
            nc.vector.memset(zf, 0.0)
            for w in range(6):
                wps = ps_tp.tile([P, KBW], f32, tag="tp", name=f"wps{w}")
                nc.tensor.matmul(
                    wps, zf[:, :P], zf, start=True, stop=True
                )

            # Persistent SBUF residents: Q^T, K^T (feature-major) and V.
            qT_t = [
                qT_pool.tile([P, DC, P], f32r, tag="qT", name=f"qT{t}")
                for t in range(NT)
            ]
            kT_b = [
                kT_pool.tile([P, DC, KBW], f32r, tag="kT", name=f"kTb{b}")
                for b in range(KB)
            ]
            vS_t = [
                vS_pool.tile([P, Db], f32r, tag="vS", name=f"vS{t}")
                for t in range(NT)
            ]

            # ---- Phase 0: load V; load + PE-transpose Q and K ----
            for t in range(NT):
                # gpsimd DMA may cast (relabel) f32 -> f32r; the PE rounds on
                # ingest anyway, so no separate rounding pass is needed
                nc.gpsimd.dma_start(vS_t[t], v[ts(t, P), :])
                nc.vector.tensor_reduce(
                    vmask[:, t : t + 1],
                    vS_t[t],
                    axis=mybir.AxisListType.X,
                    op=ALU.max,
                    apply_absolute_value=True,
                )
                kf = stage_pool.tile([P, Db], f32, tag="kf", name=f"kf{t}")
                nc.sync.dma_start(kf, k[ts(t, P), :])
                qf = stage_pool.tile([P, Db], f32, tag="qf", name=f"qf{t}")
                nc.sync.dma_start(qf, q[ts(t, P), :])
                # all DC transposes of one input tile land in ONE psum bank,
                # drained by a single wide copy (PE isn't gated on per-chunk
                # copy latency that way)
                tpk = ps_tp.tile([P, DC, P], f32, tag="tp", name=f"tpk{t}")
                tpq = ps_tp.tile([P, DC, P], f32, tag="tp", name=f"tpq{t}")
                for dc in range(DC):
                    nc.tensor.transpose(tpk[:, dc, :], kf[:, ts(dc, P)], ident)
                    nc.tensor.transpose(tpq[:, dc, :], qf[:, ts(dc, P)], ident)
                nc.scalar.copy(kT_b[t // KT][:, :, ts(t % KT, P)], tpk)
                nc.vector.tensor_copy(qT_t[t][:, :, :], tpq)
            # mask[k] = (max_d |v[k,d]|) > 0  ->  {0.0, 1.0}; pm = mask * c
            nc.vector.tensor_scalar(vmask, vmask, 0.0, None, op0=ALU.is_gt)
            pm = cpool.tile([P, NT], f32, name="pm")
            nc.vector.tensor_scalar_mul(pm, vmask, C)

            # ---- Main flash loop over k blocks ----
            for kb in range(KB):
                nums = [
                    ps_num.tile([P, Db], f32, tag=f"num{kt}", name=f"num{kb}_{kt}")
                    for kt in range(KT)
                ]
                acc = acc_pool.tile([P, KBW], f32, tag="acc", name=f"acc{kb}")
                e_tiles = {}
                # software pipeline: stage-1 (scores+exp) runs one q-tile
                # ahead of stage-2 (E^T @ V) so the PE never waits on ACT
                for qt in range(NT + 1):
                    if qt < NT:
                        s_ps = ps_s.tile([P, KBW], f32, tag="s", name=f"s{kb}_{qt}")
                        for dc in range(DC):
                            nc.tensor.matmul(
                                s_ps,
                                qT_t[qt][:, dc, :],
                                kT_b[kb][:, dc, :],
                                start=(dc == 0),
                                stop=(dc == DC - 1),
                            )
                        e = e_pool.tile([P, KBW], f32r, tag="e", name=f"e{kb}_{qt}")
                        nc.scalar.activation(e, s_ps, AF.Exp)
                        if qt == 0:
                            nc.vector.tensor_copy(acc, e)
                        else:
                            nc.vector.tensor_add(acc, acc, e)
                        e_tiles[qt] = e
                    if qt >= 1:
                        ep = e_tiles.pop(qt - 1)
                        for kt in range(KT):
                            nc.tensor.matmul(
                                nums[kt],
                                ep[:, ts(kt, P)],
                                vS_t[qt - 1],
                                start=(qt - 1 == 0),
                                stop=(qt - 1 == NT - 1),
                            )
                # close the k block: den, scale, writeback
                for kt in range(KT):
                    j = kb * KT + kt
                    dps = ps_tp.tile([P, 1], f32, tag="tp", name=f"dps{j}")
                    nc.tensor.matmul(
                        dps, acc[:, ts(kt, P)], ones, start=True, stop=True
                    )
                    # scale = pm / (den * pm + EPS), pm = mask/sqrt(D)
                    scl = sc_pool.tile([P, 1], f32, tag="scl", name=f"scl{j}")
                    nc.vector.tensor_scalar(
                        scl, dps, pm[:, j : j + 1], EPS, op0=ALU.mult, op1=ALU.add
                    )
                    rcp = sc_pool.tile([P, 1], f32, tag="rcp", name=f"rcp{j}")
                    nc.vector.reciprocal(rcp, scl)
                    nc.vector.tensor_mul(rcp, rcp, pm[:, j : j + 1])
                    o = out_pool.tile([P, Db], f32, tag="o", name=f"o{j}")
                    nc.scalar.mul(o, nums[kt], rcp)
                    nc.sync.dma_start(out[ts(j, P), :], o)

    return nc


_cache = {}


def _get_compiled(Lb=L, Db=D):
    key = (Lb, Db)
    if key not in _cache:
        nc = build_program(Lb, Db)
        nc.compile()
        _cache[key] = nc
    return _cache[key]


def run(q, k, v, trace=False):
    nc = _get_compiled()
    q = np.ascontiguousarray(q, dtype=np.float32)
    k = np.ascontiguousarray(k, dtype=np.float32)
    v = np.ascontiguousarray(v, dtype=np.float32)
    in_maps = [
        {"q": q[i], "k": k[i], "v": v[i]} for i in range(N_CORES)
    ]
    res = run_bass_kernel_spmd(nc, in_maps, list(range(N_CORES)), trace=trace)
    out = np.stack([res.results[i]["out"] for i in range(N_CORES)], axis=0)
    return out.astype(np.float32, copy=False), res


def kernel(q, k, v):
    out, _ = run(q, k, v, trace=False)
    return out


# revision 11
# speedup vs baseline: 1.0577x; 1.0077x over previous
"""Bass/Tile TRN2 kernel for nn_LzScaleDotAttention (B=8, L=2048, D=512).

Math per batch b:
    S[q,k]   = sum_d Q[q,d] K[k,d]
    E        = exp(S)                       # inputs are pre-scaled small, no max-sub needed
    num[k,d] = sum_q E[q,k] V[q,d]          # = E^T @ V
    den[k]   = sum_q E[q,k]
    mask[k]  = 1.0 if any(V[k,:] != 0) else 0.0
    out[k,d] = num[k,d] * mask[k]*c / (den[k]*mask[k]*c + EPS),  c = 1/sqrt(D)

The renormalisation over the query axis commutes with the E^T@V contraction
(the divisor depends only on k), so the normalised attention matrix is never
materialised: one flash-style pass over q tiles accumulates num (PSUM) and
den (SBUF f32 accumulator + a tiny cross-partition matmul against ones).

Sharding: batch dim (8) across the 8 NeuronCores, one batch per core (SPMD,
no collectives). Matmuls run in float32r (fp32 storage, ~1 cycle/row on the
PE for N=512). Q and K are laid out feature-major ([D, L]) host-side when
sharding, so the device spends no PE cycles transposing operands.
"""

import math
import os
import sys

import numpy as np

for _p in ("/opt/trn_rl_repo", "/root/.axon_site/_ro/trn_rl_repo"):
    if os.path.isdir(_p) and _p not in sys.path:
        sys.path.append(_p)

import concourse.bacc as bacc
import concourse.mybir as mybir
import concourse.tile as tile
from concourse.bass import ds, ts
from concourse.bass_utils import run_bass_kernel_spmd
from concourse.masks import make_identity

B, L, D = 8, 2048, 512
P = 128
EPS = 1e-7
N_CORES = 8

f32 = mybir.dt.float32
f32r = mybir.dt.float32r
AF = mybir.ActivationFunctionType
ALU = mybir.AluOpType


def build_program(Lb=L, Db=D, n_cores=N_CORES):
    """Device program. Inputs: qT, kT feature-major [D, L]; v natural [L, D]."""
    NT = Lb // P          # 128-row tiles along q / k timesteps
    DC = Db // P          # 128-wide chunks of the feature dim
    KBW = 512             # k-block width (one PSUM bank of fp32)
    KB = Lb // KBW        # k blocks
    KT = KBW // P         # 128-wide k tiles per block
    QC = Lb // KBW        # 512-wide column chunks of qT
    C = 1.0 / math.sqrt(Db)

    nc = bacc.Bacc(
        "TRN2", target_bir_lowering=False, debug=False, num_devices=n_cores
    )
    qT = nc.dram_tensor("qT", [Db, Lb], f32r, kind="ExternalInput").ap()
    kT = nc.dram_tensor("kT", [Db, Lb], f32r, kind="ExternalInput").ap()
    v = nc.dram_tensor("v", [Lb, Db], f32r, kind="ExternalInput").ap()
    out = nc.dram_tensor("out", [Lb, Db], f32, kind="ExternalOutput").ap()

    with tile.TileContext(nc) as tc:
        with (
            tc.tile_pool(name="const", bufs=1) as cpool,
            tc.tile_pool(name="qTp", bufs=DC * QC) as qT_pool,
            tc.tile_pool(name="kTp", bufs=DC * KB) as kT_pool,
            tc.tile_pool(name="vSp", bufs=NT) as vS_pool,
            tc.tile_pool(name="warm", bufs=1) as warm_pool,
            tc.tile_pool(name="ep", bufs=3) as e_pool,
            tc.tile_pool(name="accp", bufs=2) as acc_pool,
            tc.tile_pool(name="outp", bufs=3) as out_pool,
            tc.tile_pool(name="scp", bufs=4) as sc_pool,
            tc.tile_pool(name="ps_s", bufs=3, space="PSUM") as ps_s,
            tc.tile_pool(name="ps_num", bufs=1, space="PSUM") as ps_num,
            tc.tile_pool(name="ps_tp", bufs=1, space="PSUM") as ps_tp,
        ):
            ones = cpool.tile([P, 1], f32, name="ones")
            nc.vector.memset(ones, 1.0)
            vmask = cpool.tile([P, NT], f32, name="vmask")

            # PE warm-up: ~4us of dummy fp32 matmuls flips the HAM clock gate
            # to full rate before real work arrives (fp32: 4 cycles/row, so a
            # handful of instructions covers the activity window)
            zf = warm_pool.tile([P, KBW], f32, name="zf")
            nc.vector.memset(zf, 0.0)
            for w in range(6):
                wps = ps_tp.tile([P, KBW], f32, tag="tp", name=f"wps{w}")
                nc.tensor.matmul(wps, zf[:, :P], zf, start=True, stop=True)

            # Persistent SBUF residents, loaded straight from DRAM.
            # q/k column-chunk tiles [128, 512]: 2KB rows, good DMA shape.
            # kT loads issue on Sync's HWDGE ring, qT on ACT's ring, v on the
            # gpsimd SWDGE ring (casting f32 -> f32r) — three rings in parallel.
            qTs = {}
            kTs = {}
            for c in range(max(QC, KB)):
                for dc in range(DC):
                    if c < KB:
                        kt_tile = kT_pool.tile(
                            [P, KBW], f32r, tag="kT", name=f"kT{dc}_{c}"
                        )
                        nc.sync.dma_start(
                            kt_tile, kT[ds(dc * P, P), ds(c * KBW, KBW)]
                        )
                        kTs[(dc, c)] = kt_tile
                    if c < QC:
                        qt_tile = qT_pool.tile(
                            [P, KBW], f32r, tag="qT", name=f"qT{dc}_{c}"
                        )
                        nc.scalar.dma_start(
                            qt_tile, qT[ds(dc * P, P), ds(c * KBW, KBW)]
                        )
                        qTs[(dc, c)] = qt_tile
            vS_t = []
            for t in range(NT):
                vt = vS_pool.tile([P, Db], f32r, tag="vS", name=f"vS{t}")
                nc.gpsimd.dma_start(vt, v[ts(t, P), :])
                vS_t.append(vt)
                nc.vector.tensor_reduce(
                    vmask[:, t : t + 1],
                    vt,
                    axis=mybir.AxisListType.X,
                    op=ALU.max,
                    apply_absolute_value=True,
                )
            # mask[k] = (max_d |v[k,d]|) > 0 -> {0.0, 1.0}; pm = mask * c
            nc.vector.tensor_scalar(vmask, vmask, 0.0, None, op0=ALU.is_gt)
            pm = cpool.tile([P, NT], f32, name="pm")
            nc.vector.tensor_scalar_mul(pm, vmask, C)

            def q_lhsT(qt, dc):
                return qTs[(dc, qt // KT)][:, ts(qt % KT, P)]

            # ---- Main flash loop over k blocks ----
            for kb in range(KB):
                nums = [
                    ps_num.tile([P, Db], f32, tag=f"num{kt}", name=f"num{kb}_{kt}")
                    for kt in range(KT)
                ]
                acc = acc_pool.tile([P, KBW], f32, tag="acc", name=f"acc{kb}")
                e_tiles = {}
                # software pipeline: stage-1 (scores+exp) runs one q-tile
                # ahead of stage-2 (E^T @ V) so the PE never waits on ACT
                for qt in range(NT + 1):
                    if qt < NT:
                        s_ps = ps_s.tile([P, KBW], f32, tag="s", name=f"s{kb}_{qt}")
                        for dc in range(DC):
                            nc.tensor.matmul(
                                s_ps,
                                q_lhsT(qt, dc),
                                kTs[(dc, kb)],
                                start=(dc == 0),
                                stop=(dc == DC - 1),
                            )
                        e = e_pool.tile([P, KBW], f32r, tag="e", name=f"e{kb}_{qt}")
                        nc.scalar.activation(e, s_ps, AF.Exp)
                        if qt == 0:
                            nc.vector.tensor_copy(acc, e)
                        else:
                            nc.vector.tensor_add(acc, acc, e)
                        e_tiles[qt] = e
                    if qt >= 1:
                        ep = e_tiles.pop(qt - 1)
                        for kt in range(KT):
                            nc.tensor.matmul(
                                nums[kt],
                                ep[:, ts(kt, P)],
                                vS_t[qt - 1],
                                start=(qt - 1 == 0),
                                stop=(qt - 1 == NT - 1),
                            )
                # close the k block: den, scale, writeback
                for kt in range(KT):
                    j = kb * KT + kt
                    dps = ps_tp.tile([P, 1], f32, tag="tp", name=f"dps{j}")
                    nc.tensor.matmul(
                        dps, acc[:, ts(kt, P)], ones, start=True, stop=True
                    )
                    # scale = pm / (den * pm + EPS), pm = mask/sqrt(D)
                    scl = sc_pool.tile([P, 1], f32, tag="scl", name=f"scl{j}")
                    nc.vector.tensor_scalar(
                        scl, dps, pm[:, j : j + 1], EPS, op0=ALU.mult, op1=ALU.add
                    )
                    rcp = sc_pool.tile([P, 1], f32, tag="rcp", name=f"rcp{j}")
                    nc.vector.reciprocal(rcp, scl)
                    nc.vector.tensor_mul(rcp, rcp, pm[:, j : j + 1])
                    o = out_pool.tile([P, Db], f32, tag="o", name=f"o{j}")
                    # alternate the final scaled copy between ACT and DVE so
                    # the k-block epilogue drains twice as fast (stage-2 of
                    # the next block waits on these reads to reuse PSUM)
                    if kt % 2 == 0:
                        nc.scalar.mul(o, nums[kt], rcp)
                    else:
                        nc.vector.tensor_scalar_mul(o, nums[kt], rcp)
                    nc.sync.dma_start(out[ts(j, P), :], o)

    return nc


_cache = {}


def _get_compiled(Lb=L, Db=D):
    key = (Lb, Db)
    if key not in _cache:
        nc = build_program(Lb, Db)
        nc.compile()
        _cache[key] = nc
    return _cache[key]


def run(q, k, v, trace=False):
    nc = _get_compiled()
    q = np.ascontiguousarray(q, dtype=np.float32)
    k = np.ascontiguousarray(k, dtype=np.float32)
    v = np.ascontiguousarray(v, dtype=np.float32)
    in_maps = [
        {
            "qT": np.ascontiguousarray(q[i].T),
            "kT": np.ascontiguousarray(k[i].T),
            "v": v[i],
        }
        for i in range(N_CORES)
    ]
    res = run_bass_kernel_spmd(nc, in_maps, list(range(N_CORES)), trace=trace)
    out = np.stack([res.results[i]["out"] for i in range(N_CORES)], axis=0)
    return out.astype(np.float32, copy=False), res


def kernel(q, k, v):
    out, _ = run(q, k, v, trace=False)
    return out


# revision 13
# speedup vs baseline: 1.0809x; 1.0220x over previous
"""Bass/Tile TRN2 kernel for nn_LzScaleDotAttention (B=8, L=2048, D=512).

Math per batch b:
    S[q,k]   = sum_d Q[q,d] K[k,d]
    E        = exp(S)                       # inputs are pre-scaled small, no max-sub needed
    num[k,d] = sum_q E[q,k] V[q,d]          # = E^T @ V
    den[k]   = sum_q E[q,k]
    mask[k]  = 1.0 if any(V[k,:] != 0) else 0.0
    out[k,d] = num[k,d] * mask[k]*c / (den[k]*mask[k]*c + EPS),  c = 1/sqrt(D)

The renormalisation over the query axis commutes with the E^T@V contraction
(the divisor depends only on k), so the normalised attention matrix is never
materialised: one flash-style pass over q tiles accumulates num (PSUM) and
den (SBUF f32 accumulator + a tiny cross-partition matmul against ones).

Sharding: batch dim (8) across the 8 NeuronCores, one batch per core (SPMD,
no collectives). Matmuls run in float32r (fp32 storage, ~1 cycle/row on the
PE for N=512). Q and K are laid out feature-major ([D, L]) host-side when
sharding, so the device spends no PE cycles transposing operands.
"""

import math
import os
import sys

import numpy as np

for _p in ("/opt/trn_rl_repo", "/root/.axon_site/_ro/trn_rl_repo"):
    if os.path.isdir(_p) and _p not in sys.path:
        sys.path.append(_p)

import concourse.bacc as bacc
import concourse.mybir as mybir
import concourse.tile as tile
from concourse.bass import ds, ts
from concourse.bass_utils import run_bass_kernel_spmd
from concourse.masks import make_identity

B, L, D = 8, 2048, 512
P = 128
EPS = 1e-7
N_CORES = 8

f32 = mybir.dt.float32
f32r = mybir.dt.float32r
AF = mybir.ActivationFunctionType
ALU = mybir.AluOpType


def build_program(Lb=L, Db=D, n_cores=N_CORES):
    """Device program. Inputs: qT, kT feature-major [D, L]; v natural [L, D]."""
    NT = Lb // P          # 128-row tiles along q / k timesteps
    DC = Db // P          # 128-wide chunks of the feature dim
    KBW = 512             # k-block width (one PSUM bank of fp32)
    KB = Lb // KBW        # k blocks
    KT = KBW // P         # 128-wide k tiles per block
    QC = Lb // KBW        # 512-wide column chunks of qT
    C = 1.0 / math.sqrt(Db)

    nc = bacc.Bacc(
        "TRN2", target_bir_lowering=False, debug=False, num_devices=n_cores
    )
    qT = nc.dram_tensor("qT", [Db, Lb], f32r, kind="ExternalInput").ap()
    kT = nc.dram_tensor("kT", [Db, Lb], f32r, kind="ExternalInput").ap()
    v = nc.dram_tensor("v", [Lb, Db], f32r, kind="ExternalInput").ap()
    out = nc.dram_tensor("out", [Lb, Db], f32, kind="ExternalOutput").ap()

    with tile.TileContext(nc) as tc:
        with (
            tc.tile_pool(name="const", bufs=1) as cpool,
            tc.tile_pool(name="qTp", bufs=DC * QC) as qT_pool,
            tc.tile_pool(name="kTp", bufs=DC * KB) as kT_pool,
            tc.tile_pool(name="vSp", bufs=NT) as vS_pool,
            tc.tile_pool(name="warm", bufs=1) as warm_pool,
            tc.tile_pool(name="ep", bufs=3) as e_pool,
            tc.tile_pool(name="accp", bufs=2) as acc_pool,
            tc.tile_pool(name="outp", bufs=3) as out_pool,
            tc.tile_pool(name="scp", bufs=4) as sc_pool,
            tc.tile_pool(name="ps_s", bufs=3, space="PSUM") as ps_s,
            tc.tile_pool(name="ps_num", bufs=1, space="PSUM") as ps_num,
            tc.tile_pool(name="ps_tp", bufs=1, space="PSUM") as ps_tp,
        ):
            ones = cpool.tile([P, 1], f32, name="ones")
            nc.vector.memset(ones, 1.0)
            vmask = cpool.tile([P, NT], f32, name="vmask")

            # PE warm-up: ~4us of dummy fp32 matmuls flips the HAM clock gate
            # to full rate before real work arrives (fp32: 4 cycles/row, so a
            # handful of instructions covers the activity window)
            zf = warm_pool.tile([P, KBW], f32, name="zf")
            nc.vector.memset(zf, 0.0)
            for w in range(6):
                wps = ps_tp.tile([P, KBW], f32, tag="tp", name=f"wps{w}")
                nc.tensor.matmul(wps, zf[:, :P], zf, start=True, stop=True)

            # Persistent SBUF residents, loaded straight from DRAM.
            # q/k column-chunk tiles [128, 512]: 2KB rows, good DMA shape.
            # kT loads issue on Sync's HWDGE ring, qT on ACT's ring, v on the
            # gpsimd SWDGE ring (casting f32 -> f32r) — three rings in parallel.
            # Each DMA ring sustains only ~120 GB/s, so tiles are assigned to
            # the three rings (Sync-HWDGE, ACT-HWDGE, gpsimd-SWDGE) in the
            # order the flash loop consumes them: k block 0 first, all of q
            # split across two rings (it gates every q-tile of k-block 0),
            # later k blocks last.
            qTs = {}
            kTs = {}

            def load_k(dc, c, eng):
                t_ = kT_pool.tile([P, KBW], f32r, tag="kT", name=f"kT{dc}_{c}")
                eng.dma_start(t_, kT[ds(dc * P, P), ds(c * KBW, KBW)])
                kTs[(dc, c)] = t_

            def load_q(dc, c, eng):
                t_ = qT_pool.tile([P, KBW], f32r, tag="qT", name=f"qT{dc}_{c}")
                eng.dma_start(t_, qT[ds(dc * P, P), ds(c * KBW, KBW)])
                qTs[(dc, c)] = t_

            vS_t = [None] * NT

            def load_v(t, eng):
                vt = vS_pool.tile([P, Db], f32r, tag="vS", name=f"vS{t}")
                eng.dma_start(vt, v[ts(t, P), :])
                vS_t[t] = vt
                nc.vector.tensor_reduce(
                    vmask[:, t : t + 1],
                    vt,
                    axis=mybir.AxisListType.X,
                    op=ALU.max,
                    apply_absolute_value=True,
                )

            # sync ring: k block 0, then q rows 256-511, then k blocks 1-3
            for dc in range(DC):
                load_k(dc, 0, nc.sync)
            for c in range(QC):
                for dc in (2, 3):
                    load_q(dc, c, nc.sync)
            for c in range(1, KB):
                for dc in range(DC):
                    load_k(dc, c, nc.sync)
            # act ring: q rows 0-255, then the v tail
            for c in range(QC):
                for dc in (0, 1):
                    load_q(dc, c, nc.scalar)
            # gpsimd ring: v head; act ring picks up the tail after q is done
            v_split = min(10, NT)
            for t in range(v_split):
                load_v(t, nc.gpsimd)
            for t in range(v_split, NT):
                load_v(t, nc.scalar)
            # mask[k] = (max_d |v[k,d]|) > 0 -> {0.0, 1.0}; pm = mask * c
            nc.vector.tensor_scalar(vmask, vmask, 0.0, None, op0=ALU.is_gt)
            pm = cpool.tile([P, NT], f32, name="pm")
            nc.vector.tensor_scalar_mul(pm, vmask, C)

            def q_lhsT(qt, dc):
                return qTs[(dc, qt // KT)][:, ts(qt % KT, P)]

            # ---- Main flash loop over k blocks ----
            for kb in range(KB):
                nums = [
                    ps_num.tile([P, Db], f32, tag=f"num{kt}", name=f"num{kb}_{kt}")
                    for kt in range(KT)
                ]
                acc = acc_pool.tile([P, KBW], f32, tag="acc", name=f"acc{kb}")
                e_tiles = {}
                # software pipeline: stage-1 (scores+exp) runs one q-tile
                # ahead of stage-2 (E^T @ V) so the PE never waits on ACT
                for qt in range(NT + 1):
                    if qt < NT:
                        s_ps = ps_s.tile([P, KBW], f32, tag="s", name=f"s{kb}_{qt}")
                        for dc in range(DC):
                            nc.tensor.matmul(
                                s_ps,
                                q_lhsT(qt, dc),
                                kTs[(dc, kb)],
                                start=(dc == 0),
                                stop=(dc == DC - 1),
                            )
                        e = e_pool.tile([P, KBW], f32r, tag="e", name=f"e{kb}_{qt}")
                        nc.scalar.activation(e, s_ps, AF.Exp)
                        if qt == 0:
                            nc.vector.tensor_copy(acc, e)
                        else:
                            nc.vector.tensor_add(acc, acc, e)
                        e_tiles[qt] = e
                    if qt >= 1:
                        ep = e_tiles.pop(qt - 1)
                        for kt in range(KT):
                            nc.tensor.matmul(
                                nums[kt],
                                ep[:, ts(kt, P)],
                                vS_t[qt - 1],
                                start=(qt - 1 == 0),
                                stop=(qt - 1 == NT - 1),
                            )
                # close the k block: den, scale, writeback
                for kt in range(KT):
                    j = kb * KT + kt
                    dps = ps_tp.tile([P, 1], f32, tag="tp", name=f"dps{j}")
                    nc.tensor.matmul(
                        dps, acc[:, ts(kt, P)], ones, start=True, stop=True
                    )
                    # scale = pm / (den * pm + EPS), pm = mask/sqrt(D)
                    scl = sc_pool.tile([P, 1], f32, tag="scl", name=f"scl{j}")
                    nc.vector.tensor_scalar(
                        scl, dps, pm[:, j : j + 1], EPS, op0=ALU.mult, op1=ALU.add
                    )
                    rcp = sc_pool.tile([P, 1], f32, tag="rcp", name=f"rcp{j}")
                    nc.vector.reciprocal(rcp, scl)
                    nc.vector.tensor_mul(rcp, rcp, pm[:, j : j + 1])
                    o = out_pool.tile([P, Db], f32, tag="o", name=f"o{j}")
                    # alternate the final scaled copy between ACT and DVE so
                    # the k-block epilogue drains twice as fast (stage-2 of
                    # the next block waits on these reads to reuse PSUM)
                    if kt % 2 == 0:
                        nc.scalar.mul(o, nums[kt], rcp)
                    else:
                        nc.vector.tensor_scalar_mul(o, nums[kt], rcp)
                    nc.sync.dma_start(out[ts(j, P), :], o)

    return nc


_cache = {}


def _get_compiled(Lb=L, Db=D):
    key = (Lb, Db)
    if key not in _cache:
        nc = build_program(Lb, Db)
        nc.compile()
        _cache[key] = nc
    return _cache[key]


def run(q, k, v, trace=False):
    nc = _get_compiled()
    q = np.ascontiguousarray(q, dtype=np.float32)
    k = np.ascontiguousarray(k, dtype=np.float32)
    v = np.ascontiguousarray(v, dtype=np.float32)
    in_maps = [
        {
            "qT": np.ascontiguousarray(q[i].T),
            "kT": np.ascontiguousarray(k[i].T),
            "v": v[i],
        }
        for i in range(N_CORES)
    ]
    res = run_bass_kernel_spmd(nc, in_maps, list(range(N_CORES)), trace=trace)
    out = np.stack([res.results[i]["out"] for i in range(N_CORES)], axis=0)
    return out.astype(np.float32, copy=False), res


def kernel(q, k, v):
    out, _ = run(q, k, v, trace=False)
    return out


# revision 15
# speedup vs baseline: 1.1097x; 1.0266x over previous
"""Bass/Tile TRN2 kernel for nn_LzScaleDotAttention (B=8, L=2048, D=512).

Math per batch b:
    S[q,k]   = sum_d Q[q,d] K[k,d]
    E        = exp(S)                       # inputs are pre-scaled small, no max-sub needed
    num[k,d] = sum_q E[q,k] V[q,d]          # = E^T @ V
    den[k]   = sum_q E[q,k]
    mask[k]  = 1.0 if any(V[k,:] != 0) else 0.0
    out[k,d] = num[k,d] * mask[k]*c / (den[k]*mask[k]*c + EPS),  c = 1/sqrt(D)

The renormalisation over the query axis commutes with the E^T@V contraction
(the divisor depends only on k), so the normalised attention matrix is never
materialised: one flash-style pass over q tiles accumulates num (PSUM) and
den (SBUF f32 accumulator + a tiny cross-partition matmul against ones).

Sharding: batch dim (8) across the 8 NeuronCores, one batch per core (SPMD,
no collectives). Matmuls run in float32r (fp32 storage, ~1 cycle/row on the
PE for N=512). Q and K are laid out feature-major ([D, L]) host-side when
sharding, so the device spends no PE cycles transposing operands.
"""

import math
import os
import sys

import numpy as np

for _p in ("/opt/trn_rl_repo", "/root/.axon_site/_ro/trn_rl_repo"):
    if os.path.isdir(_p) and _p not in sys.path:
        sys.path.append(_p)

import concourse.bacc as bacc
import concourse.mybir as mybir
import concourse.tile as tile
from concourse.bass import ds, ts
from concourse.bass_utils import run_bass_kernel_spmd
from concourse.masks import make_identity

B, L, D = 8, 2048, 512
P = 128
EPS = 1e-7
N_CORES = 8

f32 = mybir.dt.float32
f32r = mybir.dt.float32r
AF = mybir.ActivationFunctionType
ALU = mybir.AluOpType


def build_program(Lb=L, Db=D, n_cores=N_CORES):
    """Device program. Inputs: qT, kT feature-major [D, L]; v natural [L, D]."""
    NT = Lb // P          # 128-row tiles along q / k timesteps
    DC = Db // P          # 128-wide chunks of the feature dim
    KBW = 512             # k-block width (one PSUM bank of fp32)
    KB = Lb // KBW        # k blocks
    KT = KBW // P         # 128-wide k tiles per block
    QC = Lb // KBW        # 512-wide column chunks of qT
    C = 1.0 / math.sqrt(Db)

    nc = bacc.Bacc(
        "TRN2", target_bir_lowering=False, debug=False, num_devices=n_cores
    )
    qT = nc.dram_tensor("qT", [Db, Lb], f32r, kind="ExternalInput").ap()
    kT = nc.dram_tensor("kT", [Db, Lb], f32r, kind="ExternalInput").ap()
    v = nc.dram_tensor("v", [Lb, Db], f32r, kind="ExternalInput").ap()
    out = nc.dram_tensor("out", [Lb, Db], f32, kind="ExternalOutput").ap()

    with tile.TileContext(nc) as tc:
        with (
            tc.tile_pool(name="const", bufs=1) as cpool,
            tc.tile_pool(name="qTp", bufs=DC * QC) as qT_pool,
            tc.tile_pool(name="kTp", bufs=DC * KB) as kT_pool,
            tc.tile_pool(name="vSp", bufs=NT) as vS_pool,
            tc.tile_pool(name="warm", bufs=1) as warm_pool,
            tc.tile_pool(name="ep", bufs=3) as e_pool,
            tc.tile_pool(name="accp", bufs=2) as acc_pool,
            tc.tile_pool(name="outp", bufs=3) as out_pool,
            tc.tile_pool(name="scp", bufs=4) as sc_pool,
            tc.tile_pool(name="ps_s", bufs=3, space="PSUM") as ps_s,
            tc.tile_pool(name="ps_num", bufs=1, space="PSUM") as ps_num,
            tc.tile_pool(name="ps_tp", bufs=1, space="PSUM") as ps_tp,
        ):
            ones = cpool.tile([P, 1], f32, name="ones")
            nc.vector.memset(ones, 1.0)
            vmask = cpool.tile([P, NT], f32, name="vmask")

            # PE warm-up: ~4us of dummy fp32 matmuls flips the HAM clock gate
            # to full rate before real work arrives (fp32: 4 cycles/row, so a
            # handful of instructions covers the activity window)
            zf = warm_pool.tile([P, KBW], f32, name="zf")
            nc.vector.memset(zf, 0.0)
            for w in range(8):
                wps = ps_tp.tile([P, KBW], f32, tag="tp", name=f"wps{w}")
                nc.tensor.matmul(wps, zf[:, :P], zf, start=True, stop=True)

            # Persistent SBUF residents, loaded straight from DRAM.
            # q/k column-chunk tiles [128, 512]: 2KB rows, good DMA shape.
            # kT loads issue on Sync's HWDGE ring, qT on ACT's ring, v on the
            # gpsimd SWDGE ring (casting f32 -> f32r) — three rings in parallel.
            # Each DMA ring sustains only ~120 GB/s, so tiles are assigned to
            # the three rings (Sync-HWDGE, ACT-HWDGE, gpsimd-SWDGE) in the
            # order the flash loop consumes them: k block 0 first, all of q
            # split across two rings (it gates every q-tile of k-block 0),
            # later k blocks last.
            qTs = {}
            kTs = {}

            def load_k(dc, c, eng):
                t_ = kT_pool.tile([P, KBW], f32r, tag="kT", name=f"kT{dc}_{c}")
                eng.dma_start(t_, kT[ds(dc * P, P), ds(c * KBW, KBW)])
                kTs[(dc, c)] = t_

            def load_q(dc, c, eng):
                t_ = qT_pool.tile([P, KBW], f32r, tag="qT", name=f"qT{dc}_{c}")
                eng.dma_start(t_, qT[ds(dc * P, P), ds(c * KBW, KBW)])
                qTs[(dc, c)] = t_

            vS_t = [None] * NT

            def load_v(t, eng):
                vt = vS_pool.tile([P, Db], f32r, tag="vS", name=f"vS{t}")
                eng.dma_start(vt, v[ts(t, P), :])
                vS_t[t] = vt
                nc.vector.tensor_reduce(
                    vmask[:, t : t + 1],
                    vt,
                    axis=mybir.AxisListType.X,
                    op=ALU.max,
                    apply_absolute_value=True,
                )

            # Both HWDGE engines share one physical ring (~230 GB/s); SWDGE
            # (gpsimd) is a second, slower ring. Order the shared ring by
            # first use in the flash loop: k block 0, all of q (gates every
            # q-tile of k-block 0), k block 1, the v tail, k blocks 2-3.
            # The v head rides the SWDGE ring in parallel.
            v_split = max(NT - 2, 0)
            for dc in range(DC):
                load_k(dc, 0, nc.sync)
            for c in range(QC):
                for dc in range(DC):
                    load_q(dc, c, nc.sync)
            if KB > 1:
                for dc in range(DC):
                    load_k(dc, 1, nc.sync)
            for t in range(v_split, NT):
                load_v(t, nc.sync)
            for c in range(2, KB):
                for dc in range(DC):
                    load_k(dc, c, nc.sync)
            for t in range(v_split):
                load_v(t, nc.gpsimd)
            # mask[k] = (max_d |v[k,d]|) > 0 -> {0.0, 1.0}; pm = mask * c
            nc.vector.tensor_scalar(vmask, vmask, 0.0, None, op0=ALU.is_gt)
            pm = cpool.tile([P, NT], f32, name="pm")
            nc.vector.tensor_scalar_mul(pm, vmask, C)

            def q_lhsT(qt, dc):
                return qTs[(dc, qt // KT)][:, ts(qt % KT, P)]

            # ---- Main flash loop over k blocks ----
            for kb in range(KB):
                nums = [
                    ps_num.tile([P, Db], f32, tag=f"num{kt}", name=f"num{kb}_{kt}")
                    for kt in range(KT)
                ]
                acc = acc_pool.tile([P, KBW], f32, tag="acc", name=f"acc{kb}")
                e_tiles = {}
                # software pipeline: stage-1 (scores+exp) runs one q-tile
                # ahead of stage-2 (E^T @ V) so the PE never waits on ACT
                for qt in range(NT + 1):
                    if qt < NT:
                        s_ps = ps_s.tile([P, KBW], f32, tag="s", name=f"s{kb}_{qt}")
                        for dc in range(DC):
                            nc.tensor.matmul(
                                s_ps,
                                q_lhsT(qt, dc),
                                kTs[(dc, kb)],
                                start=(dc == 0),
                                stop=(dc == DC - 1),
                            )
                        e = e_pool.tile([P, KBW], f32r, tag="e", name=f"e{kb}_{qt}")
                        nc.scalar.activation(e, s_ps, AF.Exp)
                        if qt == 0:
                            nc.vector.tensor_copy(acc, e)
                        else:
                            nc.vector.tensor_add(acc, acc, e)
                        e_tiles[qt] = e
                    if qt >= 1:
                        ep = e_tiles.pop(qt - 1)
                        for kt in range(KT):
                            nc.tensor.matmul(
                                nums[kt],
                                ep[:, ts(kt, P)],
                                vS_t[qt - 1],
                                start=(qt - 1 == 0),
                                stop=(qt - 1 == NT - 1),
                            )
                # close the k block: den, scale, writeback
                for kt in range(KT):
                    j = kb * KT + kt
                    dps = ps_tp.tile([P, 1], f32, tag="tp", name=f"dps{j}")
                    nc.tensor.matmul(
                        dps, acc[:, ts(kt, P)], ones, start=True, stop=True
                    )
                    # scale = pm / (den * pm + EPS), pm = mask/sqrt(D)
                    scl = sc_pool.tile([P, 1], f32, tag="scl", name=f"scl{j}")
                    nc.vector.tensor_scalar(
                        scl, dps, pm[:, j : j + 1], EPS, op0=ALU.mult, op1=ALU.add
                    )
                    rcp = sc_pool.tile([P, 1], f32, tag="rcp", name=f"rcp{j}")
                    nc.vector.reciprocal(rcp, scl)
                    nc.vector.tensor_mul(rcp, rcp, pm[:, j : j + 1])
                    o = out_pool.tile([P, Db], f32, tag="o", name=f"o{j}")
                    # alternate the final scaled copy between ACT and DVE so
                    # the k-block epilogue drains twice as fast (stage-2 of
                    # the next block waits on these reads to reuse PSUM)
                    if kt % 2 == 0:
                        nc.scalar.mul(o, nums[kt], rcp)
                    else:
                        nc.vector.tensor_scalar_mul(o, nums[kt], rcp)
                    nc.sync.dma_start(out[ts(j, P), :], o)

    return nc


_cache = {}


def _get_compiled(Lb=L, Db=D):
    key = (Lb, Db)
    if key not in _cache:
        nc = build_program(Lb, Db)
        nc.compile()
        _cache[key] = nc
    return _cache[key]


def run(q, k, v, trace=False):
    nc = _get_compiled()
    q = np.ascontiguousarray(q, dtype=np.float32)
    k = np.ascontiguousarray(k, dtype=np.float32)
    v = np.ascontiguousarray(v, dtype=np.float32)
    in_maps = [
        {
            "qT": np.ascontiguousarray(q[i].T),
            "kT": np.ascontiguousarray(k[i].T),
            "v": v[i],
        }
        for i in range(N_CORES)
    ]
    res = run_bass_kernel_spmd(nc, in_maps, list(range(N_CORES)), trace=trace)
    out = np.stack([res.results[i]["out"] for i in range(N_CORES)], axis=0)
    return out.astype(np.float32, copy=False), res


def kernel(q, k, v):
    out, _ = run(q, k, v, trace=False)
    return out


# revision 17
# speedup vs baseline: 1.1281x; 1.0166x over previous
"""Bass/Tile TRN2 kernel for nn_LzScaleDotAttention (B=8, L=2048, D=512).

Math per batch b:
    S[q,k]   = sum_d Q[q,d] K[k,d]
    E        = exp(S)                       # inputs are pre-scaled small, no max-sub needed
    num[k,d] = sum_q E[q,k] V[q,d]          # = E^T @ V
    den[k]   = sum_q E[q,k]
    mask[k]  = 1.0 if any(V[k,:] != 0) else 0.0
    out[k,d] = num[k,d] * mask[k]*c / (den[k]*mask[k]*c + EPS),  c = 1/sqrt(D)

The renormalisation over the query axis commutes with the E^T@V contraction
(the divisor depends only on k), so the normalised attention matrix is never
materialised: one flash-style pass over q tiles accumulates num (PSUM) and
den (SBUF f32 accumulator + a tiny cross-partition matmul against ones).

Sharding: batch dim (8) across the 8 NeuronCores, one batch per core (SPMD,
no collectives). Matmuls run in float32r (fp32 storage, ~1 cycle/row on the
PE for N=512). Q and K are laid out feature-major ([D, L]) host-side when
sharding, so the device spends no PE cycles transposing operands.
"""

import math
import os
import sys

import numpy as np

for _p in ("/opt/trn_rl_repo", "/root/.axon_site/_ro/trn_rl_repo"):
    if os.path.isdir(_p) and _p not in sys.path:
        sys.path.append(_p)

import concourse.bacc as bacc
import concourse.mybir as mybir
import concourse.tile as tile
from concourse.bass import ds, ts
from concourse.bass_utils import run_bass_kernel_spmd
from concourse.masks import make_identity

B, L, D = 8, 2048, 512
P = 128
EPS = 1e-7
N_CORES = 8

f32 = mybir.dt.float32
f32r = mybir.dt.float32r
AF = mybir.ActivationFunctionType
ALU = mybir.AluOpType


def build_program(Lb=L, Db=D, n_cores=N_CORES):
    """Device program. Inputs: qT, kT feature-major [D, L]; v natural [L, D]."""
    NT = Lb // P          # 128-row tiles along q / k timesteps
    DC = Db // P          # 128-wide chunks of the feature dim
    KBW = 512             # k-block width (one PSUM bank of fp32)
    KB = Lb // KBW        # k blocks
    KT = KBW // P         # 128-wide k tiles per block
    QC = Lb // KBW        # 512-wide column chunks of qT
    C = 1.0 / math.sqrt(Db)

    nc = bacc.Bacc(
        "TRN2", target_bir_lowering=False, debug=False, num_devices=n_cores
    )
    qT = nc.dram_tensor("qT", [Db, Lb], f32r, kind="ExternalInput").ap()
    kT = nc.dram_tensor("kT", [Db, Lb], f32r, kind="ExternalInput").ap()
    v = nc.dram_tensor("v", [Lb, Db], f32r, kind="ExternalInput").ap()
    out = nc.dram_tensor("out", [Lb, Db], f32, kind="ExternalOutput").ap()

    with tile.TileContext(nc) as tc:
        with (
            tc.tile_pool(name="const", bufs=1) as cpool,
            tc.tile_pool(name="qTp", bufs=DC * QC) as qT_pool,
            tc.tile_pool(name="kTp", bufs=DC * KB) as kT_pool,
            tc.tile_pool(name="vSp", bufs=NT) as vS_pool,
            tc.tile_pool(name="warm", bufs=1) as warm_pool,
            tc.tile_pool(name="ep", bufs=3) as e_pool,
            tc.tile_pool(name="accp", bufs=2) as acc_pool,
            tc.tile_pool(name="outp", bufs=3) as out_pool,
            tc.tile_pool(name="scp", bufs=4) as sc_pool,
            tc.tile_pool(name="ps_s", bufs=3, space="PSUM") as ps_s,
            tc.tile_pool(name="ps_num", bufs=1, space="PSUM") as ps_num,
            tc.tile_pool(name="ps_tp", bufs=1, space="PSUM") as ps_tp,
        ):
            ones = cpool.tile([P, 1], f32, name="ones")
            nc.vector.memset(ones, 1.0)
            vmask = cpool.tile([P, NT], f32, name="vmask")

            # PE warm-up: ~4us of dummy fp32 matmuls flips the HAM clock gate
            # to full rate before real work arrives (fp32: 4 cycles/row, so a
            # handful of instructions covers the activity window)
            zf = warm_pool.tile([P, KBW], f32, name="zf")
            nc.vector.memset(zf, 0.0)
            for w in range(12):
                wps = ps_tp.tile([P, KBW], f32, tag="tp", name=f"wps{w}")
                nc.tensor.matmul(wps, zf[:, :P], zf, start=True, stop=True)

            # Persistent SBUF residents, loaded straight from DRAM.
            # q/k column-chunk tiles [128, 512]: 2KB rows, good DMA shape.
            # kT loads issue on Sync's HWDGE ring, qT on ACT's ring, v on the
            # gpsimd SWDGE ring (casting f32 -> f32r) — three rings in parallel.
            # Each DMA ring sustains only ~120 GB/s, so tiles are assigned to
            # the three rings (Sync-HWDGE, ACT-HWDGE, gpsimd-SWDGE) in the
            # order the flash loop consumes them: k block 0 first, all of q
            # split across two rings (it gates every q-tile of k-block 0),
            # later k blocks last.
            qTs = {}
            kTs = {}

            def load_k(dc, c, eng):
                t_ = kT_pool.tile([P, KBW], f32r, tag="kT", name=f"kT{dc}_{c}")
                eng.dma_start(t_, kT[ds(dc * P, P), ds(c * KBW, KBW)])
                kTs[(dc, c)] = t_

            def load_q(dc, c, eng):
                t_ = qT_pool.tile([P, KBW], f32r, tag="qT", name=f"qT{dc}_{c}")
                eng.dma_start(t_, qT[ds(dc * P, P), ds(c * KBW, KBW)])
                qTs[(dc, c)] = t_

            vS_t = [None] * NT

            def load_v(t, eng):
                vt = vS_pool.tile([P, Db], f32r, tag="vS", name=f"vS{t}")
                eng.dma_start(vt, v[ts(t, P), :])
                vS_t[t] = vt
                nc.vector.tensor_reduce(
                    vmask[:, t : t + 1],
                    vt,
                    axis=mybir.AxisListType.X,
                    op=ALU.max,
                    apply_absolute_value=True,
                )

            # Both HWDGE engines share one physical ring (~230 GB/s); SWDGE
            # (gpsimd) is a second, slower ring. Order the shared ring by
            # first use in the flash loop: k block 0, all of q (gates every
            # q-tile of k-block 0), k block 1, the v tail, k blocks 2-3.
            # The v head rides the SWDGE ring in parallel.
            v_split = max(NT - 2, 0)
            for dc in range(DC):
                load_k(dc, 0, nc.sync)
            for c in range(min(2, QC)):
                for dc in range(DC):
                    load_q(dc, c, nc.sync)
            if KB > 1:
                for dc in range(DC):
                    load_k(dc, 1, nc.sync)
            for c in range(2, QC):
                for dc in range(DC):
                    load_q(dc, c, nc.sync)
            for t in range(v_split, NT):
                load_v(t, nc.sync)
            for c in range(2, KB):
                for dc in range(DC):
                    load_k(dc, c, nc.sync)
            for t in range(v_split):
                load_v(t, nc.gpsimd)
            # mask[k] = (max_d |v[k,d]|) > 0 -> {0.0, 1.0}; pm = mask * c
            nc.vector.tensor_scalar(vmask, vmask, 0.0, None, op0=ALU.is_gt)
            pm = cpool.tile([P, NT], f32, name="pm")
            nc.vector.tensor_scalar_mul(pm, vmask, C)

            def q_lhsT(qt, dc):
                return qTs[(dc, qt // KT)][:, ts(qt % KT, P)]

            # ---- Main flash loop over k blocks ----
            for kb in range(KB):
                nums = [
                    ps_num.tile([P, Db], f32, tag=f"num{kt}", name=f"num{kb}_{kt}")
                    for kt in range(KT)
                ]
                acc = acc_pool.tile([P, KBW], f32, tag="acc", name=f"acc{kb}")
                e_tiles = {}
                # software pipeline: stage-1 (scores+exp) runs one q-tile
                # ahead of stage-2 (E^T @ V) so the PE never waits on ACT
                for qt in range(NT + 1):
                    if qt < NT:
                        s_ps = ps_s.tile([P, KBW], f32, tag="s", name=f"s{kb}_{qt}")
                        for dc in range(DC):
                            nc.tensor.matmul(
                                s_ps,
                                q_lhsT(qt, dc),
                                kTs[(dc, kb)],
                                start=(dc == 0),
                                stop=(dc == DC - 1),
                            )
                        e = e_pool.tile([P, KBW], f32r, tag="e", name=f"e{kb}_{qt}")
                        nc.scalar.activation(e, s_ps, AF.Exp)
                        if qt == 0:
                            nc.vector.tensor_copy(acc, e)
                        else:
                            nc.vector.tensor_add(acc, acc, e)
                        e_tiles[qt] = e
                    if qt >= 1:
                        ep = e_tiles.pop(qt - 1)
                        for kt in range(KT):
                            nc.tensor.matmul(
                                nums[kt],
                                ep[:, ts(kt, P)],
                                vS_t[qt - 1],
                                start=(qt - 1 == 0),
                                stop=(qt - 1 == NT - 1),
                            )
                # close the k block: den, scale, writeback
                for kt in range(KT):
                    j = kb * KT + kt
                    dps = ps_tp.tile([P, 1], f32, tag="tp", name=f"dps{j}")
                    nc.tensor.matmul(
                        dps, acc[:, ts(kt, P)], ones, start=True, stop=True
                    )
                    # scale = pm / (den * pm + EPS), pm = mask/sqrt(D)
                    scl = sc_pool.tile([P, 1], f32, tag="scl", name=f"scl{j}")
                    nc.vector.tensor_scalar(
                        scl, dps, pm[:, j : j + 1], EPS, op0=ALU.mult, op1=ALU.add
                    )
                    rcp = sc_pool.tile([P, 1], f32, tag="rcp", name=f"rcp{j}")
                    nc.vector.reciprocal(rcp, scl)
                    nc.vector.tensor_mul(rcp, rcp, pm[:, j : j + 1])
                    o = out_pool.tile([P, Db], f32, tag="o", name=f"o{j}")
                    # alternate the final scaled copy between ACT and DVE so
                    # the k-block epilogue drains twice as fast (stage-2 of
                    # the next block waits on these reads to reuse PSUM)
                    if kt % 2 == 0:
                        nc.scalar.mul(o, nums[kt], rcp)
                    else:
                        nc.vector.tensor_scalar_mul(o, nums[kt], rcp)
                    nc.sync.dma_start(out[ts(j, P), :], o)

    return nc


_cache = {}


def _get_compiled(Lb=L, Db=D):
    key = (Lb, Db)
    if key not in _cache:
        nc = build_program(Lb, Db)
        nc.compile()
        _cache[key] = nc
    return _cache[key]


def run(q, k, v, trace=False):
    nc = _get_compiled()
    q = np.ascontiguousarray(q, dtype=np.float32)
    k = np.ascontiguousarray(k, dtype=np.float32)
    v = np.ascontiguousarray(v, dtype=np.float32)
    in_maps = [
        {
            "qT": np.ascontiguousarray(q[i].T),
            "kT": np.ascontiguousarray(k[i].T),
            "v": v[i],
        }
        for i in range(N_CORES)
    ]
    res = run_bass_kernel_spmd(nc, in_maps, list(range(N_CORES)), trace=trace)
    out = np.stack([res.results[i]["out"] for i in range(N_CORES)], axis=0)
    return out.astype(np.float32, copy=False), res


def kernel(q, k, v):
    out, _ = run(q, k, v, trace=False)
    return out


# revision 18
# speedup vs baseline: 1.1378x; 1.0086x over previous
"""Bass/Tile TRN2 kernel for nn_LzScaleDotAttention (B=8, L=2048, D=512).

Math per batch b:
    S[q,k]   = sum_d Q[q,d] K[k,d]
    E        = exp(S)                       # inputs are pre-scaled small, no max-sub needed
    num[k,d] = sum_q E[q,k] V[q,d]          # = E^T @ V
    den[k]   = sum_q E[q,k]
    mask[k]  = 1.0 if any(V[k,:] != 0) else 0.0
    out[k,d] = num[k,d] * mask[k]*c / (den[k]*mask[k]*c + EPS),  c = 1/sqrt(D)

The renormalisation over the query axis commutes with the E^T@V contraction
(the divisor depends only on k), so the normalised attention matrix is never
materialised: one flash-style pass over q tiles accumulates num (PSUM) and
den (SBUF f32 accumulator + a tiny cross-partition matmul against ones).

Sharding: batch dim (8) across the 8 NeuronCores, one batch per core (SPMD,
no collectives). Matmuls run in float32r (fp32 storage, ~1 cycle/row on the
PE for N=512). Q and K are laid out feature-major ([D, L]) host-side when
sharding, so the device spends no PE cycles transposing operands.
"""

import math
import os
import sys

import numpy as np

for _p in ("/opt/trn_rl_repo", "/root/.axon_site/_ro/trn_rl_repo"):
    if os.path.isdir(_p) and _p not in sys.path:
        sys.path.append(_p)

import concourse.bacc as bacc
import concourse.mybir as mybir
import concourse.tile as tile
from concourse.bass import ds, ts
from concourse.bass_utils import run_bass_kernel_spmd
from concourse.masks import make_identity

B, L, D = 8, 2048, 512
P = 128
EPS = 1e-7
N_CORES = 8

f32 = mybir.dt.float32
f32r = mybir.dt.float32r
AF = mybir.ActivationFunctionType
ALU = mybir.AluOpType


def build_program(Lb=L, Db=D, n_cores=N_CORES):
    """Device program. Inputs: qT, kT feature-major [D, L]; v natural [L, D]."""
    NT = Lb // P          # 128-row tiles along q / k timesteps
    DC = Db // P          # 128-wide chunks of the feature dim
    KBW = 512             # k-block width (one PSUM bank of fp32)
    KB = Lb // KBW        # k blocks
    KT = KBW // P         # 128-wide k tiles per block
    QC = Lb // KBW        # 512-wide column chunks of qT
    C = 1.0 / math.sqrt(Db)

    nc = bacc.Bacc(
        "TRN2", target_bir_lowering=False, debug=False, num_devices=n_cores
    )
    qT = nc.dram_tensor("qT", [Db, Lb], f32r, kind="ExternalInput").ap()
    kT = nc.dram_tensor("kT", [Db, Lb], f32r, kind="ExternalInput").ap()
    v = nc.dram_tensor("v", [Lb, Db], f32r, kind="ExternalInput").ap()
    out = nc.dram_tensor("out", [Lb, Db], f32, kind="ExternalOutput").ap()

    with tile.TileContext(nc) as tc:
        with (
            tc.tile_pool(name="const", bufs=1) as cpool,
            tc.tile_pool(name="qTp", bufs=DC * QC) as qT_pool,
            tc.tile_pool(name="kTp", bufs=DC * KB) as kT_pool,
            tc.tile_pool(name="vSp", bufs=NT) as vS_pool,
            tc.tile_pool(name="warm", bufs=1) as warm_pool,
            tc.tile_pool(name="ep", bufs=3) as e_pool,
            tc.tile_pool(name="accp", bufs=2) as acc_pool,
            tc.tile_pool(name="outp", bufs=3) as out_pool,
            tc.tile_pool(name="scp", bufs=4) as sc_pool,
            tc.tile_pool(name="ps_s", bufs=3, space="PSUM") as ps_s,
            tc.tile_pool(name="ps_num", bufs=1, space="PSUM") as ps_num,
            tc.tile_pool(name="ps_tp", bufs=1, space="PSUM") as ps_tp,
        ):
            ones = cpool.tile([P, 1], f32, name="ones")
            nc.vector.memset(ones, 1.0)
            vmask = cpool.tile([P, NT], f32, name="vmask")

            # PE warm-up: ~4us of dummy fp32 matmuls flips the HAM clock gate
            # to full rate before real work arrives (fp32: 4 cycles/row, so a
            # handful of instructions covers the activity window)
            zf = warm_pool.tile([P, KBW], f32, name="zf")
            nc.vector.memset(zf, 0.0)
            for w in range(18):
                wps = ps_tp.tile([P, KBW], f32, tag="tp", name=f"wps{w}")
                nc.tensor.matmul(wps, zf[:, :P], zf, start=True, stop=True)

            # Persistent SBUF residents, loaded straight from DRAM.
            # q/k column-chunk tiles [128, 512]: 2KB rows, good DMA shape.
            # kT loads issue on Sync's HWDGE ring, qT on ACT's ring, v on the
            # gpsimd SWDGE ring (casting f32 -> f32r) — three rings in parallel.
            # Each DMA ring sustains only ~120 GB/s, so tiles are assigned to
            # the three rings (Sync-HWDGE, ACT-HWDGE, gpsimd-SWDGE) in the
            # order the flash loop consumes them: k block 0 first, all of q
            # split across two rings (it gates every q-tile of k-block 0),
            # later k blocks last.
            qTs = {}
            kTs = {}

            def load_k(dc, c, eng):
                t_ = kT_pool.tile([P, KBW], f32r, tag="kT", name=f"kT{dc}_{c}")
                eng.dma_start(t_, kT[ds(dc * P, P), ds(c * KBW, KBW)])
                kTs[(dc, c)] = t_

            def load_q(dc, c, eng):
                t_ = qT_pool.tile([P, KBW], f32r, tag="qT", name=f"qT{dc}_{c}")
                eng.dma_start(t_, qT[ds(dc * P, P), ds(c * KBW, KBW)])
                qTs[(dc, c)] = t_

            vS_t = [None] * NT

            def load_v(t, eng):
                vt = vS_pool.tile([P, Db], f32r, tag="vS", name=f"vS{t}")
                eng.dma_start(vt, v[ts(t, P), :])
                vS_t[t] = vt
                nc.vector.tensor_reduce(
                    vmask[:, t : t + 1],
                    vt,
                    axis=mybir.AxisListType.X,
                    op=ALU.max,
                    apply_absolute_value=True,
                )

            # Both HWDGE engines share one physical ring (~230 GB/s); SWDGE
            # (gpsimd) is a second, slower ring. Order the shared ring by
            # first use in the flash loop: k block 0, all of q (gates every
            # q-tile of k-block 0), k block 1, the v tail, k blocks 2-3.
            # The v head rides the SWDGE ring in parallel.
            v_split = max(NT - 4, 0)
            for dc in range(DC):
                load_k(dc, 0, nc.sync)
            for c in range(min(2, QC)):
                for dc in range(DC):
                    load_q(dc, c, nc.sync)
            if KB > 1:
                for dc in range(DC):
                    load_k(dc, 1, nc.sync)
            for c in range(2, QC):
                for dc in range(DC):
                    load_q(dc, c, nc.sync)
            for t in range(v_split, NT):
                load_v(t, nc.sync)
            for c in range(2, KB):
                for dc in range(DC):
                    load_k(dc, c, nc.sync)
            for t in range(v_split):
                load_v(t, nc.gpsimd)
            # mask[k] = (max_d |v[k,d]|) > 0 -> {0.0, 1.0}; pm = mask * c
            nc.vector.tensor_scalar(vmask, vmask, 0.0, None, op0=ALU.is_gt)
            pm = cpool.tile([P, NT], f32, name="pm")
            nc.vector.tensor_scalar_mul(pm, vmask, C)

            def q_lhsT(qt, dc):
                return qTs[(dc, qt // KT)][:, ts(qt % KT, P)]

            # ---- Main flash loop over k blocks ----
            for kb in range(KB):
                nums = [
                    ps_num.tile([P, Db], f32, tag=f"num{kt}", name=f"num{kb}_{kt}")
                    for kt in range(KT)
                ]
                acc = acc_pool.tile([P, KBW], f32, tag="acc", name=f"acc{kb}")
                e_tiles = {}
                # software pipeline: stage-1 (scores+exp) runs one q-tile
                # ahead of stage-2 (E^T @ V) so the PE never waits on ACT
                for qt in range(NT + 1):
                    if qt < NT:
                        s_ps = ps_s.tile([P, KBW], f32, tag="s", name=f"s{kb}_{qt}")
                        for dc in range(DC):
                            nc.tensor.matmul(
                                s_ps,
                                q_lhsT(qt, dc),
                                kTs[(dc, kb)],
                                start=(dc == 0),
                                stop=(dc == DC - 1),
                            )
                        e = e_pool.tile([P, KBW], f32r, tag="e", name=f"e{kb}_{qt}")
                        nc.scalar.activation(e, s_ps, AF.Exp)
                        if qt == 0:
                            nc.vector.tensor_copy(acc, e)
                        else:
                            nc.vector.tensor_add(acc, acc, e)
                        e_tiles[qt] = e
                    if qt >= 1:
                        ep = e_tiles.pop(qt - 1)
                        for kt in range(KT):
                            nc.tensor.matmul(
                                nums[kt],
                                ep[:, ts(kt, P)],
                                vS_t[qt - 1],
                                start=(qt - 1 == 0),
                                stop=(qt - 1 == NT - 1),
                            )
                # close the k block: den, scale, writeback
                for kt in range(KT):
                    j = kb * KT + kt
                    dps = ps_tp.tile([P, 1], f32, tag="tp", name=f"dps{j}")
                    nc.tensor.matmul(
                        dps, acc[:, ts(kt, P)], ones, start=True, stop=True
                    )
                    # scale = pm / (den * pm + EPS), pm = mask/sqrt(D)
                    scl = sc_pool.tile([P, 1], f32, tag="scl", name=f"scl{j}")
                    nc.vector.tensor_scalar(
                        scl, dps, pm[:, j : j + 1], EPS, op0=ALU.mult, op1=ALU.add
                    )
                    rcp = sc_pool.tile([P, 1], f32, tag="rcp", name=f"rcp{j}")
                    nc.vector.reciprocal(rcp, scl)
                    nc.vector.tensor_mul(rcp, rcp, pm[:, j : j + 1])
                    o = out_pool.tile([P, Db], f32, tag="o", name=f"o{j}")
                    # alternate the final scaled copy between ACT and DVE so
                    # the k-block epilogue drains twice as fast (stage-2 of
                    # the next block waits on these reads to reuse PSUM)
                    if kt % 2 == 0:
                        nc.scalar.mul(o, nums[kt], rcp)
                    else:
                        nc.vector.tensor_scalar_mul(o, nums[kt], rcp)
                    nc.sync.dma_start(out[ts(j, P), :], o)

    return nc


_cache = {}


def _get_compiled(Lb=L, Db=D):
    key = (Lb, Db)
    if key not in _cache:
        nc = build_program(Lb, Db)
        nc.compile()
        _cache[key] = nc
    return _cache[key]


def run(q, k, v, trace=False):
    nc = _get_compiled()
    q = np.ascontiguousarray(q, dtype=np.float32)
    k = np.ascontiguousarray(k, dtype=np.float32)
    v = np.ascontiguousarray(v, dtype=np.float32)
    in_maps = [
        {
            "qT": np.ascontiguousarray(q[i].T),
            "kT": np.ascontiguousarray(k[i].T),
            "v": v[i],
        }
        for i in range(N_CORES)
    ]
    res = run_bass_kernel_spmd(nc, in_maps, list(range(N_CORES)), trace=trace)
    out = np.stack([res.results[i]["out"] for i in range(N_CORES)], axis=0)
    return out.astype(np.float32, copy=False), res


def kernel(q, k, v):
    out, _ = run(q, k, v, trace=False)
    return out


# revision 19
# speedup vs baseline: 1.1593x; 1.0189x over previous
"""Bass/Tile TRN2 kernel for nn_LzScaleDotAttention (B=8, L=2048, D=512).

Math per batch b:
    S[q,k]   = sum_d Q[q,d] K[k,d]
    E        = exp(S)                       # inputs are pre-scaled small, no max-sub needed
    num[k,d] = sum_q E[q,k] V[q,d]          # = E^T @ V
    den[k]   = sum_q E[q,k]
    mask[k]  = 1.0 if any(V[k,:] != 0) else 0.0
    out[k,d] = num[k,d] * mask[k]*c / (den[k]*mask[k]*c + EPS),  c = 1/sqrt(D)

The renormalisation over the query axis commutes with the E^T@V contraction
(the divisor depends only on k), so the normalised attention matrix is never
materialised: one flash-style pass over q tiles accumulates num (PSUM) and
den (SBUF f32 accumulator + a tiny cross-partition matmul against ones).

Sharding: batch dim (8) across the 8 NeuronCores, one batch per core (SPMD,
no collectives). Matmuls run in float32r (fp32 storage, ~1 cycle/row on the
PE for N=512). Q and K are laid out feature-major ([D, L]) host-side when
sharding, so the device spends no PE cycles transposing operands.
"""

import math
import os
import sys

import numpy as np

for _p in ("/opt/trn_rl_repo", "/root/.axon_site/_ro/trn_rl_repo"):
    if os.path.isdir(_p) and _p not in sys.path:
        sys.path.append(_p)

import concourse.bacc as bacc
import concourse.mybir as mybir
import concourse.tile as tile
from concourse.bass import ds, ts
from concourse.bass_utils import run_bass_kernel_spmd
from concourse.masks import make_identity

B, L, D = 8, 2048, 512
P = 128
EPS = 1e-7
N_CORES = 8

f32 = mybir.dt.float32
f32r = mybir.dt.float32r
bf16 = mybir.dt.bfloat16
AF = mybir.ActivationFunctionType
ALU = mybir.AluOpType


def build_program(Lb=L, Db=D, n_cores=N_CORES):
    """Device program. Inputs: qT, kT feature-major [D, L]; v natural [L, D]."""
    NT = Lb // P          # 128-row tiles along q / k timesteps
    DC = Db // P          # 128-wide chunks of the feature dim
    KBW = 512             # k-block width (one PSUM bank of fp32)
    KB = Lb // KBW        # k blocks
    KT = KBW // P         # 128-wide k tiles per block
    QC = Lb // KBW        # 512-wide column chunks of qT
    C = 1.0 / math.sqrt(Db)

    nc = bacc.Bacc(
        "TRN2", target_bir_lowering=False, debug=False, num_devices=n_cores
    )
    qT = nc.dram_tensor("qT", [Db, Lb], bf16, kind="ExternalInput").ap()
    kT = nc.dram_tensor("kT", [Db, Lb], bf16, kind="ExternalInput").ap()
    v = nc.dram_tensor("v", [Lb, Db], f32r, kind="ExternalInput").ap()
    out = nc.dram_tensor("out", [Lb, Db], f32, kind="ExternalOutput").ap()

    with tile.TileContext(nc) as tc:
        with (
            tc.tile_pool(name="const", bufs=1) as cpool,
            tc.tile_pool(name="qTp", bufs=DC * QC) as qT_pool,
            tc.tile_pool(name="kTp", bufs=DC * KB) as kT_pool,
            tc.tile_pool(name="vSp", bufs=NT) as vS_pool,
            tc.tile_pool(name="warm", bufs=1) as warm_pool,
            tc.tile_pool(name="ep", bufs=3) as e_pool,
            tc.tile_pool(name="accp", bufs=2) as acc_pool,
            tc.tile_pool(name="outp", bufs=3) as out_pool,
            tc.tile_pool(name="scp", bufs=4) as sc_pool,
            tc.tile_pool(name="ps_s", bufs=3, space="PSUM") as ps_s,
            tc.tile_pool(name="ps_num", bufs=1, space="PSUM") as ps_num,
            tc.tile_pool(name="ps_tp", bufs=1, space="PSUM") as ps_tp,
        ):
            ones = cpool.tile([P, 1], f32, name="ones")
            nc.vector.memset(ones, 1.0)
            vmask = cpool.tile([P, NT], f32, name="vmask")

            # PE warm-up: ~4us of dummy fp32 matmuls flips the HAM clock gate
            # to full rate before real work arrives (fp32: 4 cycles/row, so a
            # handful of instructions covers the activity window)
            zf = warm_pool.tile([P, KBW], f32, name="zf")
            nc.vector.memset(zf, 0.0)
            for w in range(18):
                wps = ps_tp.tile([P, KBW], f32, tag="tp", name=f"wps{w}")
                nc.tensor.matmul(wps, zf[:, :P], zf, start=True, stop=True)

            # Persistent SBUF residents, loaded straight from DRAM.
            # q/k column-chunk tiles [128, 512]: 2KB rows, good DMA shape.
            # kT loads issue on Sync's HWDGE ring, qT on ACT's ring, v on the
            # gpsimd SWDGE ring (casting f32 -> f32r) — three rings in parallel.
            # Each DMA ring sustains only ~120 GB/s, so tiles are assigned to
            # the three rings (Sync-HWDGE, ACT-HWDGE, gpsimd-SWDGE) in the
            # order the flash loop consumes them: k block 0 first, all of q
            # split across two rings (it gates every q-tile of k-block 0),
            # later k blocks last.
            qTs = {}
            kTs = {}

            def load_k(dc, c, eng):
                t_ = kT_pool.tile([P, KBW], bf16, tag="kT", name=f"kT{dc}_{c}")
                eng.dma_start(t_, kT[ds(dc * P, P), ds(c * KBW, KBW)])
                kTs[(dc, c)] = t_

            def load_q(dc, c, eng):
                t_ = qT_pool.tile([P, KBW], bf16, tag="qT", name=f"qT{dc}_{c}")
                eng.dma_start(t_, qT[ds(dc * P, P), ds(c * KBW, KBW)])
                qTs[(dc, c)] = t_

            vS_t = [None] * NT

            def load_v(t, eng):
                vt = vS_pool.tile([P, Db], f32r, tag="vS", name=f"vS{t}")
                eng.dma_start(vt, v[ts(t, P), :])
                vS_t[t] = vt
                nc.vector.tensor_reduce(
                    vmask[:, t : t + 1],
                    vt,
                    axis=mybir.AxisListType.X,
                    op=ALU.max,
                    apply_absolute_value=True,
                )

            # Both HWDGE engines share one physical ring (~230 GB/s); SWDGE
            # (gpsimd) is a second, slower ring. Order the shared ring by
            # first use in the flash loop: k block 0, all of q (gates every
            # q-tile of k-block 0), k block 1, the v tail, k blocks 2-3.
            # The v head rides the SWDGE ring in parallel.
            v_split = max(NT - 4, 0)
            for dc in range(DC):
                load_k(dc, 0, nc.sync)
            for c in range(min(2, QC)):
                for dc in range(DC):
                    load_q(dc, c, nc.sync)
            if KB > 1:
                for dc in range(DC):
                    load_k(dc, 1, nc.sync)
            for c in range(2, QC):
                for dc in range(DC):
                    load_q(dc, c, nc.sync)
            for t in range(v_split, NT):
                load_v(t, nc.sync)
            for c in range(2, KB):
                for dc in range(DC):
                    load_k(dc, c, nc.sync)
            for t in range(v_split):
                load_v(t, nc.gpsimd)
            # mask[k] = (max_d |v[k,d]|) > 0 -> {0.0, 1.0}; pm = mask * c
            nc.vector.tensor_scalar(vmask, vmask, 0.0, None, op0=ALU.is_gt)
            pm = cpool.tile([P, NT], f32, name="pm")
            nc.vector.tensor_scalar_mul(pm, vmask, C)

            def q_lhsT(qt, dc):
                return qTs[(dc, qt // KT)][:, ts(qt % KT, P)]

            # ---- Main flash loop over k blocks ----
            for kb in range(KB):
                nums = [
                    ps_num.tile([P, Db], f32, tag=f"num{kt}", name=f"num{kb}_{kt}")
                    for kt in range(KT)
                ]
                acc = acc_pool.tile([P, KBW], f32, tag="acc", name=f"acc{kb}")
                e_tiles = {}
                # software pipeline: stage-1 (scores+exp) runs one q-tile
                # ahead of stage-2 (E^T @ V) so the PE never waits on ACT
                for qt in range(NT + 1):
                    if qt < NT:
                        s_ps = ps_s.tile([P, KBW], f32, tag="s", name=f"s{kb}_{qt}")
                        for dc in range(DC):
                            nc.tensor.matmul(
                                s_ps,
                                q_lhsT(qt, dc),
                                kTs[(dc, kb)],
                                start=(dc == 0),
                                stop=(dc == DC - 1),
                            )
                        e = e_pool.tile([P, KBW], f32r, tag="e", name=f"e{kb}_{qt}")
                        nc.scalar.activation(e, s_ps, AF.Exp)
                        if qt == 0:
                            nc.vector.tensor_copy(acc, e)
                        else:
                            nc.vector.tensor_add(acc, acc, e)
                        e_tiles[qt] = e
                    if qt >= 1:
                        ep = e_tiles.pop(qt - 1)
                        for kt in range(KT):
                            nc.tensor.matmul(
                                nums[kt],
                                ep[:, ts(kt, P)],
                                vS_t[qt - 1],
                                start=(qt - 1 == 0),
                                stop=(qt - 1 == NT - 1),
                            )
                # close the k block: den, scale, writeback
                for kt in range(KT):
                    j = kb * KT + kt
                    dps = ps_tp.tile([P, 1], f32, tag="tp", name=f"dps{j}")
                    nc.tensor.matmul(
                        dps, acc[:, ts(kt, P)], ones, start=True, stop=True
                    )
                    # scale = pm / (den * pm + EPS), pm = mask/sqrt(D)
                    scl = sc_pool.tile([P, 1], f32, tag="scl", name=f"scl{j}")
                    nc.vector.tensor_scalar(
                        scl, dps, pm[:, j : j + 1], EPS, op0=ALU.mult, op1=ALU.add
                    )
                    rcp = sc_pool.tile([P, 1], f32, tag="rcp", name=f"rcp{j}")
                    nc.vector.reciprocal(rcp, scl)
                    nc.vector.tensor_mul(rcp, rcp, pm[:, j : j + 1])
                    o = out_pool.tile([P, Db], f32, tag="o", name=f"o{j}")
                    # alternate the final scaled copy between ACT and DVE so
                    # the k-block epilogue drains twice as fast (stage-2 of
                    # the next block waits on these reads to reuse PSUM)
                    if kt % 2 == 0:
                        nc.scalar.mul(o, nums[kt], rcp)
                    else:
                        nc.vector.tensor_scalar_mul(o, nums[kt], rcp)
                    nc.sync.dma_start(out[ts(j, P), :], o)

    return nc


_cache = {}


def _get_compiled(Lb=L, Db=D):
    key = (Lb, Db)
    if key not in _cache:
        nc = build_program(Lb, Db)
        nc.compile()
        _cache[key] = nc
    return _cache[key]


def run(q, k, v, trace=False):
    nc = _get_compiled()
    q = np.ascontiguousarray(q, dtype=np.float32)
    k = np.ascontiguousarray(k, dtype=np.float32)
    v = np.ascontiguousarray(v, dtype=np.float32)
    import ml_dtypes

    in_maps = [
        {
            "qT": np.ascontiguousarray(q[i].T).astype(ml_dtypes.bfloat16),
            "kT": np.ascontiguousarray(k[i].T).astype(ml_dtypes.bfloat16),
            "v": v[i],
        }
        for i in range(N_CORES)
    ]
    res = run_bass_kernel_spmd(nc, in_maps, list(range(N_CORES)), trace=trace)
    out = np.stack([res.results[i]["out"] for i in range(N_CORES)], axis=0)
    return out.astype(np.float32, copy=False), res


def kernel(q, k, v):
    out, _ = run(q, k, v, trace=False)
    return out


# revision 20
# speedup vs baseline: 1.1781x; 1.0162x over previous
"""Bass/Tile TRN2 kernel for nn_LzScaleDotAttention (B=8, L=2048, D=512).

Math per batch b:
    S[q,k]   = sum_d Q[q,d] K[k,d]
    E        = exp(S)                       # inputs are pre-scaled small, no max-sub needed
    num[k,d] = sum_q E[q,k] V[q,d]          # = E^T @ V
    den[k]   = sum_q E[q,k]
    mask[k]  = 1.0 if any(V[k,:] != 0) else 0.0
    out[k,d] = num[k,d] * mask[k]*c / (den[k]*mask[k]*c + EPS),  c = 1/sqrt(D)

The renormalisation over the query axis commutes with the E^T@V contraction
(the divisor depends only on k), so the normalised attention matrix is never
materialised: one flash-style pass over q tiles accumulates num (PSUM) and
den (SBUF f32 accumulator + a tiny cross-partition matmul against ones).

Sharding: batch dim (8) across the 8 NeuronCores, one batch per core (SPMD,
no collectives). Matmuls run in float32r (fp32 storage, ~1 cycle/row on the
PE for N=512). Q and K are laid out feature-major ([D, L]) host-side when
sharding, so the device spends no PE cycles transposing operands.
"""

import math
import os
import sys

import numpy as np

for _p in ("/opt/trn_rl_repo", "/root/.axon_site/_ro/trn_rl_repo"):
    if os.path.isdir(_p) and _p not in sys.path:
        sys.path.append(_p)

import concourse.bacc as bacc
import concourse.mybir as mybir
import concourse.tile as tile
from concourse.bass import ds, ts
from concourse.bass_utils import run_bass_kernel_spmd
from concourse.masks import make_identity

B, L, D = 8, 2048, 512
P = 128
EPS = 1e-7
N_CORES = 8

f32 = mybir.dt.float32
f32r = mybir.dt.float32r
bf16 = mybir.dt.bfloat16
AF = mybir.ActivationFunctionType
ALU = mybir.AluOpType


def build_program(Lb=L, Db=D, n_cores=N_CORES):
    """Device program. Inputs: qT, kT feature-major [D, L]; v natural [L, D]."""
    NT = Lb // P          # 128-row tiles along q / k timesteps
    DC = Db // P          # 128-wide chunks of the feature dim
    KBW = 512             # k-block width (one PSUM bank of fp32)
    KB = Lb // KBW        # k blocks
    KT = KBW // P         # 128-wide k tiles per block
    QC = Lb // KBW        # 512-wide column chunks of qT
    C = 1.0 / math.sqrt(Db)

    nc = bacc.Bacc(
        "TRN2", target_bir_lowering=False, debug=False, num_devices=n_cores
    )
    qT = nc.dram_tensor("qT", [Db, Lb], bf16, kind="ExternalInput").ap()
    kT = nc.dram_tensor("kT", [Db, Lb], bf16, kind="ExternalInput").ap()
    v = nc.dram_tensor("v", [Lb, Db], f32r, kind="ExternalInput").ap()
    out = nc.dram_tensor("out", [Lb, Db], f32, kind="ExternalOutput").ap()

    with tile.TileContext(nc) as tc:
        with (
            tc.tile_pool(name="const", bufs=1) as cpool,
            tc.tile_pool(name="qTp", bufs=DC * QC) as qT_pool,
            tc.tile_pool(name="kTp", bufs=DC * KB) as kT_pool,
            tc.tile_pool(name="vSp", bufs=NT) as vS_pool,
            tc.tile_pool(name="warm", bufs=1) as warm_pool,
            tc.tile_pool(name="ep", bufs=3) as e_pool,
            tc.tile_pool(name="accp", bufs=2) as acc_pool,
            tc.tile_pool(name="outp", bufs=3) as out_pool,
            tc.tile_pool(name="scp", bufs=4) as sc_pool,
            tc.tile_pool(name="ps_s", bufs=3, space="PSUM") as ps_s,
            tc.tile_pool(name="ps_num", bufs=1, space="PSUM") as ps_num,
            tc.tile_pool(name="ps_tp", bufs=1, space="PSUM") as ps_tp,
        ):
            ones = cpool.tile([P, 1], f32, name="ones")
            nc.vector.memset(ones, 1.0)
            vmask = cpool.tile([P, NT], f32, name="vmask")

            # PE warm-up: ~4us of dummy fp32 matmuls flips the HAM clock gate
            # to full rate before real work arrives (fp32: 4 cycles/row, so a
            # handful of instructions covers the activity window)
            zf = warm_pool.tile([P, KBW], f32, name="zf")
            nc.vector.memset(zf, 0.0)
            wps = ps_tp.tile([P, KBW], f32, tag="tp", name="wps")
            for w in range(10):
                # all into one psum tile: pure WAW chain, no pool churn
                nc.tensor.matmul(wps, zf[:, :P], zf, start=True, stop=True)

            # Persistent SBUF residents, loaded straight from DRAM.
            # q/k column-chunk tiles [128, 512]: 2KB rows, good DMA shape.
            # kT loads issue on Sync's HWDGE ring, qT on ACT's ring, v on the
            # gpsimd SWDGE ring (casting f32 -> f32r) — three rings in parallel.
            # Each DMA ring sustains only ~120 GB/s, so tiles are assigned to
            # the three rings (Sync-HWDGE, ACT-HWDGE, gpsimd-SWDGE) in the
            # order the flash loop consumes them: k block 0 first, all of q
            # split across two rings (it gates every q-tile of k-block 0),
            # later k blocks last.
            qTs = {}
            kTs = {}

            def load_k(dc, c, eng):
                t_ = kT_pool.tile([P, KBW], bf16, tag="kT", name=f"kT{dc}_{c}")
                eng.dma_start(t_, kT[ds(dc * P, P), ds(c * KBW, KBW)])
                kTs[(dc, c)] = t_

            def load_q(dc, c, eng):
                t_ = qT_pool.tile([P, KBW], bf16, tag="qT", name=f"qT{dc}_{c}")
                eng.dma_start(t_, qT[ds(dc * P, P), ds(c * KBW, KBW)])
                qTs[(dc, c)] = t_

            vS_t = [None] * NT

            def load_v(t, eng):
                vt = vS_pool.tile([P, Db], f32r, tag="vS", name=f"vS{t}")
                eng.dma_start(vt, v[ts(t, P), :])
                vS_t[t] = vt
                nc.vector.tensor_reduce(
                    vmask[:, t : t + 1],
                    vt,
                    axis=mybir.AxisListType.X,
                    op=ALU.max,
                    apply_absolute_value=True,
                )

            # Both HWDGE engines share one physical ring (~230 GB/s); SWDGE
            # (gpsimd) is a second, slower ring. Order the shared ring by
            # first use in the flash loop: k block 0, all of q (gates every
            # q-tile of k-block 0), k block 1, the v tail, k blocks 2-3.
            # The v head rides the SWDGE ring in parallel.
            v_split = max(NT - 8, 0)
            for dc in range(DC):
                load_k(dc, 0, nc.sync)
            for c in range(min(2, QC)):
                for dc in range(DC):
                    load_q(dc, c, nc.sync)
            if KB > 1:
                for dc in range(DC):
                    load_k(dc, 1, nc.sync)
            for c in range(2, QC):
                for dc in range(DC):
                    load_q(dc, c, nc.sync)
            for t in range(v_split, NT):
                load_v(t, nc.sync)
            for c in range(2, KB):
                for dc in range(DC):
                    load_k(dc, c, nc.sync)
            for t in range(v_split):
                load_v(t, nc.gpsimd)
            # mask[k] = (max_d |v[k,d]|) > 0 -> {0.0, 1.0}; pm = mask * c
            nc.vector.tensor_scalar(vmask, vmask, 0.0, None, op0=ALU.is_gt)
            pm = cpool.tile([P, NT], f32, name="pm")
            nc.vector.tensor_scalar_mul(pm, vmask, C)

            def q_lhsT(qt, dc):
                return qTs[(dc, qt // KT)][:, ts(qt % KT, P)]

            # ---- Main flash loop over k blocks ----
            for kb in range(KB):
                nums = [
                    ps_num.tile([P, Db], f32, tag=f"num{kt}", name=f"num{kb}_{kt}")
                    for kt in range(KT)
                ]
                acc = acc_pool.tile([P, KBW], f32, tag="acc", name=f"acc{kb}")
                e_tiles = {}
                # software pipeline: stage-1 (scores+exp) runs one q-tile
                # ahead of stage-2 (E^T @ V) so the PE never waits on ACT
                for qt in range(NT + 1):
                    if qt < NT:
                        s_ps = ps_s.tile([P, KBW], f32, tag="s", name=f"s{kb}_{qt}")
                        for dc in range(DC):
                            nc.tensor.matmul(
                                s_ps,
                                q_lhsT(qt, dc),
                                kTs[(dc, kb)],
                                start=(dc == 0),
                                stop=(dc == DC - 1),
                            )
                        e = e_pool.tile([P, KBW], f32r, tag="e", name=f"e{kb}_{qt}")
                        nc.scalar.activation(e, s_ps, AF.Exp)
                        if qt == 0:
                            nc.vector.tensor_copy(acc, e)
                        else:
                            nc.vector.tensor_add(acc, acc, e)
                        e_tiles[qt] = e
                    if qt >= 1:
                        ep = e_tiles.pop(qt - 1)
                        for kt in range(KT):
                            nc.tensor.matmul(
                                nums[kt],
                                ep[:, ts(kt, P)],
                                vS_t[qt - 1],
                                start=(qt - 1 == 0),
                                stop=(qt - 1 == NT - 1),
                            )
                # close the k block: den, scale, writeback
                for kt in range(KT):
                    j = kb * KT + kt
                    dps = ps_tp.tile([P, 1], f32, tag="tp", name=f"dps{j}")
                    nc.tensor.matmul(
                        dps, acc[:, ts(kt, P)], ones, start=True, stop=True
                    )
                    # scale = pm / (den * pm + EPS), pm = mask/sqrt(D)
                    scl = sc_pool.tile([P, 1], f32, tag="scl", name=f"scl{j}")
                    nc.vector.tensor_scalar(
                        scl, dps, pm[:, j : j + 1], EPS, op0=ALU.mult, op1=ALU.add
                    )
                    rcp = sc_pool.tile([P, 1], f32, tag="rcp", name=f"rcp{j}")
                    nc.vector.reciprocal(rcp, scl)
                    nc.vector.tensor_mul(rcp, rcp, pm[:, j : j + 1])
                    o = out_pool.tile([P, Db], f32, tag="o", name=f"o{j}")
                    # alternate the final scaled copy between ACT and DVE so
                    # the k-block epilogue drains twice as fast (stage-2 of
                    # the next block waits on these reads to reuse PSUM)
                    if kt % 2 == 0:
                        nc.scalar.mul(o, nums[kt], rcp)
                    else:
                        nc.vector.tensor_scalar_mul(o, nums[kt], rcp)
                    nc.sync.dma_start(out[ts(j, P), :], o)

    return nc


_cache = {}


def _get_compiled(Lb=L, Db=D):
    key = (Lb, Db)
    if key not in _cache:
        nc = build_program(Lb, Db)
        nc.compile()
        _cache[key] = nc
    return _cache[key]


def run(q, k, v, trace=False):
    nc = _get_compiled()
    q = np.ascontiguousarray(q, dtype=np.float32)
    k = np.ascontiguousarray(k, dtype=np.float32)
    v = np.ascontiguousarray(v, dtype=np.float32)
    import ml_dtypes

    in_maps = [
        {
            "qT": np.ascontiguousarray(q[i].T).astype(ml_dtypes.bfloat16),
            "kT": np.ascontiguousarray(k[i].T).astype(ml_dtypes.bfloat16),
            "v": v[i],
        }
        for i in range(N_CORES)
    ]
    res = run_bass_kernel_spmd(nc, in_maps, list(range(N_CORES)), trace=trace)
    out = np.stack([res.results[i]["out"] for i in range(N_CORES)], axis=0)
    return out.astype(np.float32, copy=False), res


def kernel(q, k, v):
    out, _ = run(q, k, v, trace=False)
    return out


# revision 21
# speedup vs baseline: 1.1903x; 1.0103x over previous
"""Bass/Tile TRN2 kernel for nn_LzScaleDotAttention (B=8, L=2048, D=512).

Math per batch b:
    S[q,k]   = sum_d Q[q,d] K[k,d]
    E        = exp(S)                       # inputs are pre-scaled small, no max-sub needed
    num[k,d] = sum_q E[q,k] V[q,d]          # = E^T @ V
    den[k]   = sum_q E[q,k]
    mask[k]  = 1.0 if any(V[k,:] != 0) else 0.0
    out[k,d] = num[k,d] * mask[k]*c / (den[k]*mask[k]*c + EPS),  c = 1/sqrt(D)

The renormalisation over the query axis commutes with the E^T@V contraction
(the divisor depends only on k), so the normalised attention matrix is never
materialised: one flash-style pass over q tiles accumulates num (PSUM) and
den (SBUF f32 accumulator + a tiny cross-partition matmul against ones).

Sharding: batch dim (8) across the 8 NeuronCores, one batch per core (SPMD,
no collectives). Matmuls run in float32r (fp32 storage, ~1 cycle/row on the
PE for N=512). Q and K are laid out feature-major ([D, L]) host-side when
sharding, so the device spends no PE cycles transposing operands.
"""

import math
import os
import sys

import numpy as np

for _p in ("/opt/trn_rl_repo", "/root/.axon_site/_ro/trn_rl_repo"):
    if os.path.isdir(_p) and _p not in sys.path:
        sys.path.append(_p)

import concourse.bacc as bacc
import concourse.mybir as mybir
import concourse.tile as tile
from concourse.bass import ds, ts
from concourse.bass_utils import run_bass_kernel_spmd
from concourse.masks import make_identity

B, L, D = 8, 2048, 512
P = 128
EPS = 1e-7
N_CORES = 8

f32 = mybir.dt.float32
f32r = mybir.dt.float32r
bf16 = mybir.dt.bfloat16
AF = mybir.ActivationFunctionType
ALU = mybir.AluOpType


def build_program(Lb=L, Db=D, n_cores=N_CORES):
    """Device program. Inputs: qT, kT feature-major [D, L]; v natural [L, D]."""
    NT = Lb // P          # 128-row tiles along q / k timesteps
    DC = Db // P          # 128-wide chunks of the feature dim
    KBW = 512             # k-block width (one PSUM bank of fp32)
    KB = Lb // KBW        # k blocks
    KT = KBW // P         # 128-wide k tiles per block
    QC = Lb // KBW        # 512-wide column chunks of qT
    C = 1.0 / math.sqrt(Db)

    nc = bacc.Bacc(
        "TRN2", target_bir_lowering=False, debug=False, num_devices=n_cores
    )
    qT = nc.dram_tensor("qT", [Db, Lb], bf16, kind="ExternalInput").ap()
    kT = nc.dram_tensor("kT", [Db, Lb], bf16, kind="ExternalInput").ap()
    v = nc.dram_tensor("v", [Lb, Db], f32r, kind="ExternalInput").ap()
    out = nc.dram_tensor("out", [Lb, Db], f32, kind="ExternalOutput").ap()

    with tile.TileContext(nc) as tc:
        with (
            tc.tile_pool(name="const", bufs=1) as cpool,
            tc.tile_pool(name="qTp", bufs=DC * QC) as qT_pool,
            tc.tile_pool(name="kTp", bufs=DC * KB) as kT_pool,
            tc.tile_pool(name="vSp", bufs=NT) as vS_pool,
            tc.tile_pool(name="warm", bufs=1) as warm_pool,
            tc.tile_pool(name="ep", bufs=3) as e_pool,
            tc.tile_pool(name="accp", bufs=2) as acc_pool,
            tc.tile_pool(name="outp", bufs=3) as out_pool,
            tc.tile_pool(name="scp", bufs=4) as sc_pool,
            tc.tile_pool(name="ps_s", bufs=3, space="PSUM") as ps_s,
            tc.tile_pool(name="ps_num", bufs=1, space="PSUM") as ps_num,
            tc.tile_pool(name="ps_tp", bufs=1, space="PSUM") as ps_tp,
        ):
            ones = cpool.tile([P, 1], f32, name="ones")
            nc.vector.memset(ones, 1.0)
            vmask = cpool.tile([P, NT], f32, name="vmask")

            # PE warm-up: ~4us of dummy fp32 matmuls flips the HAM clock gate
            # to full rate before real work arrives (fp32: 4 cycles/row, so a
            # handful of instructions covers the activity window)
            zf = warm_pool.tile([P, KBW], f32, name="zf")
            nc.vector.memset(zf, 0.0)
            wps = ps_tp.tile([P, KBW], f32, tag="tp", name="wps")
            for w in range(10):
                # all into one psum tile: pure WAW chain, no pool churn
                nc.tensor.matmul(wps, zf[:, :P], zf, start=True, stop=True)

            # Persistent SBUF residents, loaded straight from DRAM.
            # q/k column-chunk tiles [128, 512]: 2KB rows, good DMA shape.
            # kT loads issue on Sync's HWDGE ring, qT on ACT's ring, v on the
            # gpsimd SWDGE ring (casting f32 -> f32r) — three rings in parallel.
            # Each DMA ring sustains only ~120 GB/s, so tiles are assigned to
            # the three rings (Sync-HWDGE, ACT-HWDGE, gpsimd-SWDGE) in the
            # order the flash loop consumes them: k block 0 first, all of q
            # split across two rings (it gates every q-tile of k-block 0),
            # later k blocks last.
            qTs = {}
            kTs = {}

            def load_k(dc, c, eng):
                t_ = kT_pool.tile([P, KBW], bf16, tag="kT", name=f"kT{dc}_{c}")
                eng.dma_start(t_, kT[ds(dc * P, P), ds(c * KBW, KBW)])
                kTs[(dc, c)] = t_

            def load_q(dc, c, eng):
                t_ = qT_pool.tile([P, KBW], bf16, tag="qT", name=f"qT{dc}_{c}")
                eng.dma_start(t_, qT[ds(dc * P, P), ds(c * KBW, KBW)])
                qTs[(dc, c)] = t_

            vS_t = [None] * NT

            def load_v(t, eng):
                vt = vS_pool.tile([P, Db], f32r, tag="vS", name=f"vS{t}")
                eng.dma_start(vt, v[ts(t, P), :])
                vS_t[t] = vt
                nc.vector.tensor_reduce(
                    vmask[:, t : t + 1],
                    vt,
                    axis=mybir.AxisListType.X,
                    op=ALU.max,
                    apply_absolute_value=True,
                )

            # Both HWDGE engines share one physical ring (~230 GB/s); SWDGE
            # (gpsimd) is a second, slower ring. Order the shared ring by
            # first use in the flash loop: k block 0, all of q (gates every
            # q-tile of k-block 0), k block 1, the v tail, k blocks 2-3.
            # The v head rides the SWDGE ring in parallel.
            v_split = max(NT - 8, 0)
            for dc in range(DC):
                load_k(dc, 0, nc.sync)
            for c in range(min(2, QC)):
                for dc in range(DC):
                    load_q(dc, c, nc.sync)
            if KB > 1:
                for dc in range(DC):
                    load_k(dc, 1, nc.sync)
            for c in range(2, QC):
                for dc in range(DC):
                    load_q(dc, c, nc.sync)
            for t in range(v_split, NT):
                load_v(t, nc.sync)
            for c in range(2, KB):
                for dc in range(DC):
                    load_k(dc, c, nc.sync)
            for t in range(v_split):
                load_v(t, nc.gpsimd)
            # mask[k] = (max_d |v[k,d]|) > 0 -> {0.0, 1.0}; pm = mask * c
            nc.vector.tensor_scalar(vmask, vmask, 0.0, None, op0=ALU.is_gt)
            pm = cpool.tile([P, NT], f32, name="pm")
            nc.vector.tensor_scalar_mul(pm, vmask, C)

            def q_lhsT(qt, dc):
                return qTs[(dc, qt // KT)][:, ts(qt % KT, P)]

            # ---- Main flash loop over k blocks ----
            # The per-block epilogue (den, scale, writeback) is emitted inside
            # the NEXT block's first q-tile so its engine work interleaves
            # with the pipeline refill instead of stalling the PE on PSUM
            # slot reuse at every block boundary.
            def make_epilogue(kb, acc, nums):
                def emit():
                    for kt in range(KT):
                        j = kb * KT + kt
                        dps = ps_tp.tile([P, 1], f32, tag="tp", name=f"dps{j}")
                        nc.tensor.matmul(
                            dps, acc[:, ts(kt, P)], ones, start=True, stop=True
                        )
                        # scale = pm / (den * pm + EPS), pm = mask/sqrt(D)
                        scl = sc_pool.tile([P, 1], f32, tag="scl", name=f"scl{j}")
                        nc.vector.tensor_scalar(
                            scl, dps, pm[:, j : j + 1], EPS,
                            op0=ALU.mult, op1=ALU.add,
                        )
                        rcp = sc_pool.tile([P, 1], f32, tag="rcp", name=f"rcp{j}")
                        nc.vector.reciprocal(rcp, scl)
                        nc.vector.tensor_mul(rcp, rcp, pm[:, j : j + 1])
                        o = out_pool.tile([P, Db], f32, tag="o", name=f"o{j}")
                        nc.vector.tensor_scalar_mul(o, nums[kt], rcp)
                        nc.sync.dma_start(out[ts(j, P), :], o)
                return emit

            pending_epilogue = None
            for kb in range(KB):
                acc = acc_pool.tile([P, KBW], f32, tag="acc", name=f"acc{kb}")
                nums = None
                e_tiles = {}
                # software pipeline: stage-1 (scores+exp) runs one q-tile
                # ahead of stage-2 (E^T @ V) so the PE never waits on ACT
                for qt in range(NT + 1):
                    if qt < NT:
                        s_ps = ps_s.tile([P, KBW], f32, tag="s", name=f"s{kb}_{qt}")
                        for dc in range(DC):
                            nc.tensor.matmul(
                                s_ps,
                                q_lhsT(qt, dc),
                                kTs[(dc, kb)],
                                start=(dc == 0),
                                stop=(dc == DC - 1),
                            )
                        e = e_pool.tile([P, KBW], f32r, tag="e", name=f"e{kb}_{qt}")
                        nc.scalar.activation(e, s_ps, AF.Exp)
                        if qt == 0 and pending_epilogue is not None:
                            # previous block's den/scale/writeback lands here,
                            # after this block's first scores+exp are queued
                            pending_epilogue()
                            pending_epilogue = None
                        if qt == 0:
                            nc.vector.tensor_copy(acc, e)
                        else:
                            nc.vector.tensor_add(acc, acc, e)
                        e_tiles[qt] = e
                    if qt >= 1:
                        if nums is None:
                            # allocate after the previous block's release ops
                            # so the pool trace sees release before alloc
                            nums = [
                                ps_num.tile(
                                    [P, Db], f32,
                                    tag=f"num{kt}", name=f"num{kb}_{kt}",
                                )
                                for kt in range(KT)
                            ]
                        ep = e_tiles.pop(qt - 1)
                        for kt in range(KT):
                            nc.tensor.matmul(
                                nums[kt],
                                ep[:, ts(kt, P)],
                                vS_t[qt - 1],
                                start=(qt - 1 == 0),
                                stop=(qt - 1 == NT - 1),
                            )
                pending_epilogue = make_epilogue(kb, acc, nums)
            pending_epilogue()

    return nc


_cache = {}


def _get_compiled(Lb=L, Db=D):
    key = (Lb, Db)
    if key not in _cache:
        nc = build_program(Lb, Db)
        nc.compile()
        _cache[key] = nc
    return _cache[key]


def run(q, k, v, trace=False):
    nc = _get_compiled()
    q = np.ascontiguousarray(q, dtype=np.float32)
    k = np.ascontiguousarray(k, dtype=np.float32)
    v = np.ascontiguousarray(v, dtype=np.float32)
    import ml_dtypes

    in_maps = [
        {
            "qT": np.ascontiguousarray(q[i].T).astype(ml_dtypes.bfloat16),
            "kT": np.ascontiguousarray(k[i].T).astype(ml_dtypes.bfloat16),
            "v": v[i],
        }
        for i in range(N_CORES)
    ]
    res = run_bass_kernel_spmd(nc, in_maps, list(range(N_CORES)), trace=trace)
    out = np.stack([res.results[i]["out"] for i in range(N_CORES)], axis=0)
    return out.astype(np.float32, copy=False), res


def kernel(q, k, v):
    out, _ = run(q, k, v, trace=False)
    return out
